# revision 40
# baseline (speedup 1.0000x reference)
"""Trainium2 Bass kernel for the hyperbolic (Poincare-ball) AddRNN NLI model.

Sharding: 8 cores SPMD. Core c: RNN role r=c//4 (0=premise, 1=hypothesis),
batch quarter q=c%4 (rows 32q..32q+31). Each core runs the full sequential
scan for its (role, quarter).

Architecture (beyond the original folded-scalar implementation):
- Folded-H layout: every wide [32,1024] tensor lives as [128,256], with
  partition 32j+b holding batch row b's H-chunk j. All wide DVE/ACT ops
  (square-accums, state update, tanh, masked accumulates) run 4x faster
  than batch-major, and the per-step state transpose is 2 full 128x128 PE
  transposes instead of 8 thin ones.
- The recurrent W-matmul runs as 4 concurrent PE column-groups
  (tile_position=(0,32j)), group j computing H cols [256j,256j+258) of
  t@W straight into its folded PSUM strip; 96 bf16 N=258 matmuls/step at
  ~110ns per 4-way group when warm. bf16 hi/lo 3-pass GEMM (th@Wh + th@Wl
  + tl@Wh) keeps per-dot error ~2^-18 (fp32r single-pass amplifies to
  ~2e-2 over 256 steps - too close to the gate). The W@b bias-dot rides
  as column 256 of group 3's weight tile.
- Cross-strip reductions (|uG|^2, <uG,y>, <uG,Wb>, |t|^2) go through tiny
  fp32 matmuls with I4rep = tile(I32,(4,4)) which both sum the 4 strips
  AND replicate to all 128 partitions, so the whole Mobius scalar chain
  runs on [128,1] tiles and its outputs feed the folded state update
  directly (no broadcast). Three decoupled partial->reduce paths let the
  chain's first Ln wait only on the |uG|^2 path.
- P1 (per-token y = mmatvec(x,U) GEMM) is interleaved into the scan: one
  128-token tile per 2 steps plus chunked table-chains, hiding ~0.45ms of
  prologue inside the scan's serial-latency gaps (Tile tracks the
  ydr DRAM RAW deps). P1 writes ydr pre-folded via a 3-level DMA AP.
- HAM keep-warm: 3 bursts of 3 fp32 dummy matmuls, data-dependent on
  mid-chain scalars, keep the PE activity monitor at K=8/8 through the
  ~9us scalar-chain window; without them the whole matmul phase runs at
  1.2GHz (measured ~5.3us vs ~2.9us warm).
- ACT tables pinned to natural_log_exp_and_others + sigmoid_and_others
  (2 hidden table loads/step). Cancellation-free gyro algebra:
  1-|x(+)y|^2 = (1-|x|^2)(1-|y|^2)/den through both madds;
  1-tanh^2 = 4rv(1-rv); den1 = A1 - y2*B1 (uses 1-th^2 == B1 exactly,
  clamps included); den2 = A2 - b2*B2; fused tensor_scalar/stt forms
  throughout; artanh via Ln(n2, bias=1) - Ln(Bz2).
- Late-chain shortcut: the wide update W2 = uG + (cycp/cu)y + (cb/cu)b is
  built MID-chain (the ratios cycp/cu = B1*cyt/aa and cb/cu =
  B2/(A2*gam*aa) need no late chain values), and cu is applied through
  the tanh's free scale slot - removing cycp->w0->w1->w2 (~1.6us) from
  the serial tail. P1 tiles are injected at the END of the step body so
  their DVE copies queue into the next chain window, not ahead of the
  reduce ops; w0's old ACT Copy is gone so the tanh table-load no longer
  stalls TANH in the ACT FIFO.
Per-step ~16.2us: matmul ~2.8 + reduce ~1.3 + scalar chain ~9 +
tanh/transpose ~2. History: 8.36ms baseline -> 5.88 (folding +
col-tiling) -> 5.25 (replicated chain, P1 interleave, fusions) -> 4.78
(HAM keep-warm bursts) -> 4.55 (cu-scale tail cut) -> 4.54ms (early lnB
so the tanh table-load starts sooner; cut = L*g1*aa drops the m2 op).
Tried+reverted: epilogue weight preload into cpool (+3.7us/step SBUF
shuffle), split CAST per transpose block (+0.75us/step).
"""
import numpy as np

B, S, E, H, V, NCLS = 128, 256, 300, 1024, 32000, 3
BL = 32
MAXN = 1.0 - 1e-3
BMIN = float(1.0 - np.float64(MAXN) * np.float64(MAXN))  # 1-MAXN^2
_A999 = float(np.arctanh(np.float64(MAXN)))

_CACHE = {}

# cst column map
C_B2, C_BB, C_BC2, C_BBC = 0, 1, 2, 3
C_PK2, C_PKAK, C_AN, C_LAM, C_BPK = 4, 7, 10, 13, 16
C_OPB2, C_NB2 = 19, 20
CSTW = 21


def _make_bacc():
    """Bacc with activation-table insertion pinned to two sets so the
    Ln/Exp/Square/Copy cluster and the wide Tanh never thrash tables."""
    from concourse import bacc
    from concourse.hw_specs import get_activation_tables
    import bass_rust as _bass_rust

    class _Bacc2(bacc.Bacc):
        def insert_act_table_loads(self):
            from concourse import mybir as mb
            has_activation = any(
                isinstance(i, mb.InstActivation)
                for b in self.main_func.blocks
                for i in b.instructions
            )
            if not has_activation:
                return
            # Keep the canonical list (act_func_set_id is positional into
            # act_info.json) but empty out every set except the two we pin,
            # so the chooser can only bind activations to those.
            tabs = get_activation_tables(self.m.arch)
            keep = ("natural_log_exp_and_others", "sigmoid_and_others")
            tables = [(k, v if k in keep else set()) for k, v in tabs.items()]
            _bass_rust.insert_act_table_loads(self, tables)

    return _Bacc2()


def _build(s_steps, use_gpsimd_sel=None):
    import os
    if use_gpsimd_sel is None:
        use_gpsimd_sel = os.environ.get("V2_GPSEL", "1") == "1"
    use_f32r = os.environ.get("V2_F32R", "1") == "1"
    use_tabs = os.environ.get("V2_TABS", "1") == "1"

    import concourse.tile as tile
    from concourse import mybir

    f32 = mybir.dt.float32
    f32r = mybir.dt.float32r if use_f32r else mybir.dt.float32
    bf16 = mybir.dt.bfloat16
    AF = mybir.ActivationFunctionType
    OP = mybir.AluOpType

    NT = BL * s_steps
    NTILES = NT // 128
    HP1 = H + 2  # weights padded to even width for fp32r ISA rules
    HP2 = H + 2  # allgather payload: rep | x2 | B

    if use_tabs:
        nc = _make_bacc()
    else:
        from concourse import bacc as _bacc
        nc = _bacc.Bacc()

    # register the Ln-bias constant as a const AP (only 0.0/1.0 are built in)
    _cb = nc.alloc_sbuf_tensor("const-f32-1em12", [128, 1], f32)
    nc.gpsimd.memset(_cb.ap(), 1e-12)
    nc.const_aps.aps[(f32, 1e-12)] = _cb.ap()
    nc.all_engine_barrier()

    GW = 258          # per-col-group W width: 256 H cols + b col + pad
    xTh = nc.declare_dram_parameter("xTh", [E, NT], bf16, isOutput=False)
    xTl = nc.declare_dram_parameter("xTl", [E, NT], bf16, isOutput=False)
    xn2d = nc.declare_dram_parameter("xn2d", [128, NTILES], f32, isOutput=False)
    Uch = nc.declare_dram_parameter("Uch", [E, HP1], bf16, isOutput=False)
    Ucl = nc.declare_dram_parameter("Ucl", [E, HP1], bf16, isOutput=False)
    Wrh = nc.declare_dram_parameter("Wrh", [H, 4 * GW], bf16, isOutput=False)
    Wrl = nc.declare_dram_parameter("Wrl", [H, 4 * GW], bf16, isOutput=False)
    Wl = nc.declare_dram_parameter("Wl", [H, HP1], f32r, isOutput=False)
    brep = nc.declare_dram_parameter("brep", [128, 256], f32, isOutput=False)
    bcrep = nc.declare_dram_parameter("bcrep", [128, H], f32, isOutput=False)
    mlrT = nc.declare_dram_parameter("mlrT", [H, 2 * NCLS], f32r, isOutput=False)
    cst = nc.declare_dram_parameter("cst", [128, CSTW], f32, isOutput=False)
    eqm = nc.declare_dram_parameter("eqm", [128, s_steps], f32, isOutput=False)
    ident = nc.declare_dram_parameter("ident", [128, 128], f32, isOutput=False)
    i4m = nc.declare_dram_parameter("i4m", [128, 128], f32, isOutput=False)
    out = nc.declare_dram_parameter("out", [B, NCLS], f32, isOutput=True)

    ydr = nc.dram_tensor("ydr", [s_steps * 128, 256], f32)
    ccin = nc.dram_tensor("ccin", [BL, HP2], f32)
    ccout = nc.dram_tensor("ccout", [8 * BL, HP2], f32, addr_space="Shared")

    with tile.TileContext(nc) as tc:
      with tc.tile_pool(name="const", bufs=1) as cpool:
        # ---------------- constants to SBUF --------------------------------
        wh_sb, wl_sb = [], []
        for ki in range(8):
            th_ = cpool.tile([128, 4 * GW], bf16, tag=f"wh{ki}", name=f"wh{ki}")
            nc.sync.dma_start(th_[:], Wrh[ki * 128:(ki + 1) * 128, :])
            wh_sb.append(th_)
            tl_ = cpool.tile([128, 4 * GW], bf16, tag=f"wl{ki}b", name=f"wl{ki}b")
            nc.sync.dma_start(tl_[:], Wrl[ki * 128:(ki + 1) * 128, :])
            wl_sb.append(tl_)
        uh_sb, ul_sb = [], []
        for c, kc in enumerate((128, 128, E - 256)):
            th_ = cpool.tile([128, HP1], bf16, tag=f"uh{c}", name=f"uh{c}")
            nc.sync.dma_start(th_[0:kc, :], Uch[c * 128:c * 128 + kc, :])
            uh_sb.append(th_)
            tl_ = cpool.tile([128, HP1], bf16, tag=f"ul{c}", name=f"ul{c}")
            nc.sync.dma_start(tl_[0:kc, :], Ucl[c * 128:c * 128 + kc, :])
            ul_sb.append(tl_)
        brep_sb = cpool.tile([128, 256], f32, tag="brep")
        nc.sync.dma_start(brep_sb[:], brep[:, :])
        bcrep_sb = cpool.tile([128, H], f32, tag="bcrep")
        nc.sync.dma_start(bcrep_sb[:], bcrep[:, :])
        cst_sb = cpool.tile([128, CSTW], f32, tag="cst")
        nc.sync.dma_start(cst_sb[:], cst[:, :])
        ident_sb = cpool.tile([128, 128], f32, tag="ident")
        nc.sync.dma_start(ident_sb[:], ident[:, :])
        i4_sb = cpool.tile([128, 128], f32, tag="i4m")
        nc.sync.dma_start(i4_sb[:], i4m[:, :])
        eqm_sb = cpool.tile([128, s_steps], f32, tag="eqm")
        nc.sync.dma_start(eqm_sb[:], eqm[:, :])
        xn2a = cpool.tile([128, NTILES], f32, tag="xn2a")
        nc.sync.dma_start(xn2a[:], xn2d[:, :])
        un2a = cpool.tile([128, NTILES], f32, tag="un2a")
        uba = cpool.tile([128, NTILES], f32, tag="uba")
        # per-step tables, replicated x4 along partitions: [128, s_steps]
        y2s = cpool.tile([128, s_steps], f32, tag="y2s")
        y2p1s = cpool.tile([128, s_steps], f32, tag="y2p1s")
        Bys = cpool.tile([128, s_steps], f32, tag="Bys")
        cys = cpool.tile([128, s_steps], f32, tag="cys")
        sybs = cpool.tile([128, s_steps], f32, tag="sybs")
        ny2s = cpool.tile([128, s_steps], f32, tag="ny2s")

        b2c = cst_sb[:, C_B2:C_B2 + 1]
        Bbc = cst_sb[:, C_BB:C_BB + 1]
        opb2c = cst_sb[:, C_OPB2:C_OPB2 + 1]
        nb2c = cst_sb[:, C_NB2:C_NB2 + 1]

        # =============== P1: prologue  y = mmatvec(x, U) ====================
        # P1 (prologue GEMM y = mmatvec(x,U)) and P2 (the scan) share one
        # pool scope: P1 tiles 8..NTILES-1 and the chunked table-chains are
        # emitted INSIDE the step loop (one tile per 2 steps), so their
        # PE/DVE/ACT work hides in the scan's serial-latency gaps. DRAM RAW
        # deps (ydr write -> yt read) are tracked by Tile, order is safe.
        with (
            tc.tile_pool(name="bigio", bufs=2) as bpool,
            tc.tile_pool(name="psA", bufs=1, space="PSUM") as psA,
            tc.tile_pool(name="scal", bufs=2) as spool,
            tc.tile_pool(name="state", bufs=1) as stp,
            tc.tile_pool(name="rbig", bufs=2) as rb,
            tc.tile_pool(name="rps", bufs=2, space="PSUM") as rps,
            tc.tile_pool(name="rpsT", bufs=1, space="PSUM") as rpsT,
            tc.tile_pool(name="rpsS", bufs=1, space="PSUM") as rpsS,
            tc.tile_pool(name="rsc", bufs=3) as rs,
        ):
            CH = ((0, 512), (512, 1024), (1024, 1026))

            def p1_tile(j):
                xkh = bpool.tile([128, 3 * 128], bf16, tag="xkh")
                nc.sync.dma_start(
                    xkh[:, 0:256].rearrange("p (c m) -> p c m", c=2),
                    xTh[0:256, j * 128:(j + 1) * 128].rearrange(
                        "(c p) m -> p c m", c=2))
                nc.sync.dma_start(
                    xkh[0:E - 256, 256:384],
                    xTh[256:E, j * 128:(j + 1) * 128])
                xkl = bpool.tile([128, 3 * 128], bf16, tag="xkl")
                nc.sync.dma_start(
                    xkl[:, 0:256].rearrange("p (c m) -> p c m", c=2),
                    xTl[0:256, j * 128:(j + 1) * 128].rearrange(
                        "(c p) m -> p c m", c=2))
                nc.sync.dma_start(
                    xkl[0:E - 256, 256:384],
                    xTl[256:E, j * 128:(j + 1) * 128])

                pu = psA.tile([128, HP1], f32, tag="pu")
                for c, kc in enumerate((128, 128, E - 256)):
                    for (n0, n1) in CH:
                        nc.tensor.matmul(
                            pu[:, n0:n1],
                            xkh[0:kc, c * 128:c * 128 + 128],
                            uh_sb[c][0:kc, n0:n1],
                            start=(c == 0), stop=False)
                    for (n0, n1) in CH:
                        nc.tensor.matmul(
                            pu[:, n0:n1],
                            xkh[0:kc, c * 128:c * 128 + 128],
                            ul_sb[c][0:kc, n0:n1],
                            start=False, stop=False)
                for c, kc in enumerate((128, 128, E - 256)):
                    for (n0, n1) in CH:
                        nc.tensor.matmul(
                            pu[:, n0:n1],
                            xkl[0:kc, c * 128:c * 128 + 128],
                            uh_sb[c][0:kc, n0:n1],
                            start=False, stop=(c == 2))
                ysc = bpool.tile([128, H], f32, tag="ysc")
                nc.vector.tensor_copy(ysc[:], pu[:, 0:H])
                nc.vector.tensor_copy(uba[:, j:j + 1], pu[:, H:H + 1])
                # folded scatter: ydr[(4j+sl)*128 + jj*32 + b, c] =
                #   y[b, 4j+sl][256*jj + c]
                for sl in range(4):
                    nc.sync.dma_start(
                        ydr[(j * 4 + sl) * 128:(j * 4 + sl + 1) * 128,
                            :].rearrange("(jj b) c -> b jj c", jj=4, b=BL),
                        ysc[sl * BL:(sl + 1) * BL, :].rearrange(
                            "b (jj c) -> b jj c", jj=4))
                scrH = bpool.tile([128, H], f32, tag="scrH")
                nc.scalar.activation(scrH[:], ysc[:], AF.Square,
                                     accum_out=un2a[:, j:j + 1])

            def p1_chain(c0, c1):
                # chunked prologue scalar chain on [128, c1-c0]
                W_ = c1 - c0

                def sc(tag):
                    return spool.tile([128, W_], f32, tag=f"{tag}_{W_}",
                                      name=f"{tag}_{c0}")

                xn = sc("p_xn")
                nc.scalar.activation(xn[:], xn2a[:, c0:c1], AF.Ln, bias=1e-12)
                nc.scalar.activation(xn[:], xn[:], AF.Exp, scale=0.5)
                rxn = sc("p_rxn")
                nc.vector.reciprocal(rxn[:], xn[:])
                pxa = sc("p_pxa")
                nc.vector.tensor_scalar(pxa[:], xn[:], 1.0, None, OP.add)
                mxa = sc("p_mxa")
                nc.vector.tensor_scalar(mxa[:], xn[:], -1.0, 1.0,
                                        OP.mult, OP.add)
                nc.vector.reciprocal(mxa[:], mxa[:])
                nc.vector.tensor_tensor(pxa[:], pxa[:], mxa[:], OP.mult)
                lnr = sc("p_lnr")
                nc.scalar.activation(lnr[:], pxa[:], AF.Ln)  # = 2*artanh(xn)
                un = sc("p_un")
                nc.scalar.activation(un[:], un2a[:, c0:c1], AF.Ln, bias=1e-12)
                nc.scalar.activation(un[:], un[:], AF.Exp, scale=0.5)
                run = sc("p_run")
                nc.vector.reciprocal(run[:], un[:])
                arg = sc("p_arg")
                nc.vector.tensor_tensor(arg[:], un[:], rxn[:], OP.mult)
                nc.vector.tensor_tensor(arg[:], arg[:], lnr[:], OP.mult)
                # arg = 2*(un/xn)*artanh(xn); tanh = 1-2/(e^arg+1)
                ev = sc("p_ev")
                nc.scalar.activation(ev[:], arg[:], AF.Exp)
                nc.vector.tensor_scalar(ev[:], ev[:], 1.0, None, OP.add)
                rv = sc("p_rv")
                nc.vector.reciprocal(rv[:], ev[:])
                th = sc("p_th")
                nc.vector.tensor_scalar(th[:], rv[:], -2.0, 1.0,
                                        OP.mult, OP.add)
                nc.vector.tensor_scalar(th[:], th[:], MAXN, None, OP.min)
                y2a = sc("p_y2a")
                nc.vector.tensor_tensor(y2a[:], th[:], th[:], OP.mult)
                y2p1a = sc("p_y2p1a")
                nc.vector.tensor_scalar(y2p1a[:], y2a[:], 1.0, None, OP.add)
                dneg = sc("p_dneg")
                nc.vector.tensor_tensor(dneg[:], rv[:], rv[:], OP.mult)
                nc.vector.tensor_tensor(dneg[:], dneg[:], rv[:], OP.subtract)
                Bya = sc("p_Bya")
                nc.vector.tensor_scalar(Bya[:], dneg[:], -4.0, BMIN,
                                        OP.mult, OP.max)
                cya = sc("p_cya")
                nc.vector.tensor_tensor(cya[:], th[:], run[:], OP.mult)
                syba = sc("p_syba")
                nc.vector.tensor_tensor(syba[:], cya[:], uba[:, c0:c1],
                                        OP.mult)
                ny2a = sc("p_ny2a")
                nc.vector.tensor_scalar(ny2a[:], y2a[:], -1.0, None, OP.mult)
                for g in range(4):
                    for jj in range(4):
                        dst = slice(jj * BL, (jj + 1) * BL)
                        sg = slice(g * BL, (g + 1) * BL)
                        cs = slice(4 * c0 + g, 4 * c1, 4)
                        nc.sync.dma_start(y2s[dst, cs], y2a[sg, :])
                        nc.sync.dma_start(y2p1s[dst, cs], y2p1a[sg, :])
                        nc.sync.dma_start(Bys[dst, cs], Bya[sg, :])
                        nc.sync.dma_start(cys[dst, cs], cya[sg, :])
                        nc.sync.dma_start(sybs[dst, cs], syba[sg, :])
                        nc.sync.dma_start(ny2s[dst, cs], ny2a[sg, :])

            # prefix: first 8 tiles + their tables (feeds steps 0..31)
            PRE = min(8, NTILES)
            for j in range(PRE):
                p1_tile(j)
            p1_chain(0, PRE)

            tTh = stp.tile([128, 256], bf16, tag="tTh")
            nc.vector.memset(tTh[:].bitcast(mybir.dt.uint16), 0)
            tTl = stp.tile([128, 256], bf16, tag="tTl")
            nc.vector.memset(tTl[:].bitcast(mybir.dt.uint16), 0)
            kk2 = stp.tile([128, 1], f32, tag="kk2")
            nc.vector.memset(kk2[:], 2.0)
            tsel = cpool.tile([128, 256], f32, tag="tsel", name="tsel")
            nc.vector.memset(tsel[:], 0.0)
            s2tsel = cpool.tile([BL, 1], f32, tag="s2tsel", name="s2tsel")
            nc.vector.memset(s2tsel[:], 0.0)

            def stat(tt, ki):
                c0 = (ki % 2) * 128 + (ki // 2) * BL
                return tt[:, c0:c0 + BL]

            def s(tag):
                return rs.tile([128, 1], f32, tag=tag, name=tag)

            next_tile = PRE
            next_chain = PRE
            for t in range(s_steps):
                yt = rb.tile([128, 256], f32, tag="yt")
                nc.sync.dma_start(yt[:], ydr[t * 128:(t + 1) * 128, :])
                pT = rpsT.tile([128, 256], f32, tag="pT")

                # --- W matmul: 4 col-groups concurrent, k-outer -----------
                pu = rps.tile([128, GW], f32, tag="rpu")
                for ki in range(8):
                    sh = stat(tTh, ki)
                    for g in range(4):
                        nc.tensor.matmul(
                            pu[g * BL:(g + 1) * BL, :], sh,
                            wh_sb[ki][:, g * GW:(g + 1) * GW],
                            start=(ki == 0), stop=False,
                            tile_position=(0, g * BL))
                        nc.tensor.matmul(
                            pu[g * BL:(g + 1) * BL, :], sh,
                            wl_sb[ki][:, g * GW:(g + 1) * GW],
                            start=False, stop=False,
                            tile_position=(0, g * BL))
                for ki in range(8):
                    sl_ = stat(tTl, ki)
                    for g in range(4):
                        nc.tensor.matmul(
                            pu[g * BL:(g + 1) * BL, :], sl_,
                            wh_sb[ki][:, g * GW:(g + 1) * GW],
                            start=False, stop=(ki == 7),
                            tile_position=(0, g * BL))

                y2 = y2s[:, t:t + 1]
                ny2 = ny2s[:, t:t + 1]
                y2p1 = y2p1s[:, t:t + 1]
                By = Bys[:, t:t + 1]
                cyt = cys[:, t:t + 1]
                syb = sybs[:, t:t + 1]

                # --- folded norms: three decoupled partial->reduce paths --
                scr = rb.tile([128, 256], f32, tag="scr")
                prtA = rb.tile([128, 1], f32, tag="prtA")
                nc.scalar.activation(scr[:], pu[:, 0:256], AF.Square,
                                     accum_out=prtA[:])
                prtC = rb.tile([128, 1], f32, tag="prtC")
                nc.vector.tensor_copy(prtC[:], pu[:, 256:257])
                scr2 = rb.tile([128, 256], f32, tag="scr2")
                prtB = rb.tile([128, 1], f32, tag="prtB")
                nc.vector.scalar_tensor_tensor(
                    scr2[:], pu[:, 0:256], 1.0, yt[:],
                    OP.mult, OP.mult, accum_out=prtB[:])
                psmA = rpsS.tile([128, 1], f32, tag="psmA")
                psmX = rpsS.tile([128, 3], f32, tag="psmX")
                nc.tensor.matmul(psmA[:], i4_sb[:], prtA[:],
                                 start=True, stop=True)
                nc.tensor.matmul(psmX[:, 1:2], i4_sb[:], prtC[:],
                                 start=True, stop=True)
                nc.tensor.matmul(psmX[:, 0:1], i4_sb[:], prtB[:],
                                 start=True, stop=True)
                s2u = psmA[:]
                suy = psmX[:, 0:1]
                sub = psmX[:, 1:2]

                # --- X-norm block -----------------------------------------
                lnu = s("lnu")
                nc.scalar.activation(lnu[:], s2u, AF.Ln, bias=1e-12)
                nG = s("nG")
                nc.scalar.activation(nG[:], lnu[:], AF.Exp, scale=0.5)
                ev = s("ev")
                nc.scalar.activation(ev[:], nG[:], AF.Exp, scale=kk2[:])
                q = s("q")
                nc.vector.tensor_scalar(q[:], ev[:], 1.0, None, OP.add)
                rv = s("rv")
                nc.vector.reciprocal(rv[:], q[:])
                th = s("th")
                nc.vector.tensor_scalar(th[:], rv[:], -2.0, 1.0, OP.mult, OP.add)
                nc.vector.tensor_scalar(th[:], th[:], MAXN, None, OP.min)
                dneg = s("dneg")
                nc.vector.scalar_tensor_tensor(
                    dneg[:], rv[:], rv[:], rv[:], OP.mult, OP.subtract)
                B1 = s("B1")
                nc.vector.tensor_scalar(B1[:], dneg[:], -4.0, BMIN,
                                        OP.mult, OP.max)
                rnG = s("rnG")
                nc.scalar.activation(rnG[:], lnu[:], AF.Exp, scale=-0.5)
                cwm = s("cwm")
                nc.vector.tensor_tensor(cwm[:], th[:], rnG[:], OP.mult)
                for db in range(3):
                    nc.tensor.matmul(pT[0:1, 0:256], cwm[0:128, 0:1],
                                     brep_sb[:, 0:256],
                                     start=True, stop=True,
                                     tile_position=(0, 0),
                                     skip_group_check=True)

                # --- madd1 scalars ----------------------------------------
                xy1 = s("xy1")
                nc.vector.tensor_scalar(xy1[:], suy, cyt, cwm[:],
                                        OP.mult, OP.mult)
                r1 = s("r1")
                nc.vector.tensor_scalar(r1[:], suy, cyt, B1[:],
                                        OP.mult, OP.mult)
                A1 = s("A1")
                nc.vector.tensor_scalar(A1[:], xy1[:], 2.0, y2p1,
                                        OP.mult, OP.add)
                den1 = s("den1")
                nc.vector.tensor_scalar(den1[:], B1[:], ny2, A1[:],
                                        OP.mult, OP.add)
                aa = s("aa")
                nc.vector.tensor_tensor(aa[:], A1[:], cwm[:], OP.mult)
                raa = s("raa")
                nc.vector.reciprocal(raa[:], aa[:])
                r1c = s("r1c")
                nc.vector.tensor_scalar(r1c[:], B1[:], cyt, raa[:],
                                        OP.mult, OP.mult)
                r2 = s("r2")
                nc.vector.tensor_scalar(r2[:], aa[:], s2u, None, OP.mult)
                r3 = s("r3")
                nc.vector.scalar_tensor_tensor(
                    r3[:], r1[:], 2.0, r2[:], OP.mult, OP.add)
                r5 = s("r5")
                nc.vector.tensor_scalar(r5[:], B1[:], B1[:], y2,
                                        OP.mult, OP.mult)
                s2n1 = s("s2n1")
                nc.vector.scalar_tensor_tensor(
                    s2n1[:], r3[:], aa[:], r5[:], OP.mult, OP.add)
                rn1 = s("rn1")
                nc.scalar.activation(rn1[:], s2n1[:], AF.Ln, bias=1e-12)
                nc.scalar.activation(rn1[:], rn1[:], AF.Exp, scale=-0.5)
                rd1 = s("rd1")
                nc.vector.reciprocal(rd1[:], den1[:])
                gam = s("gam")
                nc.vector.tensor_scalar(gam[:], rn1[:], MAXN, rd1[:],
                                        OP.mult, OP.min)
                for db in range(3):
                    nc.tensor.matmul(pT[0:1, 0:256], gam[0:128, 0:1],
                                     brep_sb[:, 0:256],
                                     start=True, stop=True,
                                     tile_position=(0, 0),
                                     skip_group_check=True)
                B2 = s("B2")
                nc.vector.tensor_scalar(B2[:], B1[:], By, rd1[:],
                                        OP.mult, OP.mult)
                nc.vector.tensor_scalar(B2[:], B2[:], BMIN, None, OP.max)
                x2b = s("x2b")
                nc.vector.tensor_scalar(x2b[:], B2[:], -1.0, 1.0,
                                        OP.mult, OP.add)

                # --- madd2 scalars (with hyperbolic bias b) ---------------
                t3 = s("t3")
                nc.vector.tensor_scalar(t3[:], aa[:], sub, None, OP.mult)
                t5 = s("t5")
                nc.vector.scalar_tensor_tensor(
                    t5[:], B1[:], syb, t3[:], OP.mult, OP.add)
                xy2d = s("xy2d")
                nc.vector.tensor_scalar(xy2d[:], t5[:], gam[:], 2.0,
                                        OP.mult, OP.mult)
                A2 = s("A2")
                nc.vector.tensor_scalar(A2[:], xy2d[:], opb2c, None, OP.add)
                den2 = s("den2")
                nc.vector.scalar_tensor_tensor(
                    den2[:], B2[:], nb2c, A2[:], OP.mult, OP.add)
                g1 = s("g1")
                nc.vector.tensor_scalar(g1[:], A2[:], gam[:], None, OP.mult)
                rg1 = s("rg1")
                nc.vector.reciprocal(rg1[:], g1[:])
                r2c = s("r2c")
                nc.vector.tensor_scalar(r2c[:], B2[:], raa[:], rg1[:],
                                        OP.mult, OP.mult)
                W1 = rb.tile([128, 256], f32, tag="W1")
                nc.vector.scalar_tensor_tensor(
                    W1[:], yt[:], r1c[:], pu[:, 0:256], OP.mult, OP.add)
                W2 = rb.tile([128, 256], f32, tag="W2")
                nc.vector.scalar_tensor_tensor(
                    W2[:], brep_sb[:], r2c[:], W1[:], OP.mult, OP.add)
                u1 = s("u1")
                nc.vector.tensor_tensor(u1[:], A2[:], x2b[:], OP.mult)
                u3 = s("u3")
                nc.vector.scalar_tensor_tensor(
                    u3[:], xy2d[:], B2[:], u1[:], OP.mult, OP.add)
                u5 = s("u5")
                nc.vector.tensor_scalar(u5[:], B2[:], B2[:], b2c,
                                        OP.mult, OP.mult)
                s2n2 = s("s2n2")
                nc.vector.scalar_tensor_tensor(
                    s2n2[:], u3[:], A2[:], u5[:], OP.mult, OP.add)
                ln2 = s("ln2")
                nc.scalar.activation(ln2[:], s2n2[:], AF.Ln, bias=1e-12)
                n2r = s("n2r")
                nc.scalar.activation(n2r[:], ln2[:], AF.Exp, scale=0.5)
                rd2 = s("rd2")
                nc.vector.reciprocal(rd2[:], den2[:])
                Bz2 = s("Bz2")
                nc.vector.tensor_scalar(Bz2[:], B2[:], Bbc, rd2[:],
                                        OP.mult, OP.mult)
                nc.vector.tensor_scalar(Bz2[:], Bz2[:], BMIN, None, OP.max)
                lnB = s("lnB")
                nc.scalar.activation(lnB[:], Bz2[:], AF.Ln)
                n2 = s("n2")
                nc.vector.tensor_scalar(n2[:], n2r[:], rd2[:], MAXN,
                                        OP.mult, OP.min)
                for db in range(3):
                    nc.tensor.matmul(pT[0:1, 0:256], n2[0:128, 0:1],
                                     brep_sb[:, 0:256],
                                     start=True, stop=True,
                                     tile_position=(0, 0),
                                     skip_group_check=True)
                lnA = s("lnA")
                nc.scalar.activation(lnA[:], n2[:], AF.Ln, bias=1.0)
                rn2r = s("rn2r")
                nc.scalar.activation(rn2r[:], ln2[:], AF.Exp, scale=-0.5)
                a2 = s("a2")
                nc.vector.scalar_tensor_tensor(
                    a2[:], lnA[:], 2.0, lnB[:], OP.mult, OP.subtract)
                L = s("L")
                nc.vector.tensor_scalar(L[:], a2[:], rn2r[:], 0.5,
                                        OP.mult, OP.mult)
                cut = s("cut")
                nc.vector.tensor_scalar(cut[:], L[:], g1[:], aa[:],
                                        OP.mult, OP.mult)

                # --- wide state update: W2 = uG + (cycp/cu)y + (cb/cu)b
                # was built mid-chain; apply cu via the tanh scale slot ----
                tv = rb.tile([128, 256], f32, tag="tv")
                nc.scalar.activation(tv[:], W2[:], AF.Tanh, scale=cut[:])
                s2tp = rb.tile([128, 1], f32, tag="s2tp")
                nc.scalar.activation(scr[:], tv[:], AF.Square,
                                     accum_out=s2tp[:])
                nc.tensor.matmul(psmX[:, 2:3], i4_sb[:], s2tp[:],
                                 start=True, stop=True)
                rnt = s("rnt")
                nc.scalar.activation(rnt[:], psmX[:, 2:3], AF.Ln,
                                     bias=1e-12)
                nc.scalar.activation(rnt[:], rnt[:], AF.Exp, scale=-0.5)
                nc.vector.tensor_scalar(kk2[:], rnt[:], 2.0 * _A999, 2.0,
                                        OP.mult, OP.min)

                # --- transpose new state (2 full 128x128 blocks) ----------
                nc.tensor.transpose(pT[:, 0:128], tv[:, 0:128], ident_sb[:])
                nc.tensor.transpose(pT[:, 128:256], tv[:, 128:256],
                                    ident_sb[:])
                nc.vector.tensor_copy(tTh[:], pT[:])
                nc.vector.tensor_tensor(tTl[:], pT[:], tTh[:], OP.subtract)

                # masked last-state accumulate (folded) - consumed only by
                # the epilogue, so it queues AFTER the critical CAST/SUB
                nc.vector.scalar_tensor_tensor(
                    tsel[:], tv[:], eqm_sb[:, t:t + 1], tsel[:],
                    OP.mult, OP.add)
                nc.vector.scalar_tensor_tensor(
                    s2tsel[:], psmX[0:BL, 2:3], eqm_sb[0:BL, t:t + 1],
                    s2tsel[:], OP.mult, OP.add)

                # inject one deferred P1 tile every other step, at the END
                # of the body so its DVE/PE work queues into the next
                # chain window instead of ahead of this step's reduce ops
                if next_tile < NTILES and t % 2 == 1:
                    p1_tile(next_tile)
                    next_tile += 1
                    if next_tile % 8 == 0 or next_tile == NTILES:
                        p1_chain(next_chain, next_tile)
                        next_chain = next_tile

        # =============== P3: epilogue =======================================
        with (
            tc.tile_pool(name="ebig", bufs=1) as eb,
            tc.tile_pool(name="eps", bufs=1, space="PSUM") as eps,
            tc.tile_pool(name="esc", bufs=2) as es,
        ):
            def e(tag):
                return es.tile([BL, 1], f32, tag=tag, name=tag)

            # last state scalars: ntl = |t_last|, tanh, B_h
            ntl = e("ntl")
            nc.scalar.activation(ntl[:], s2tsel[:], AF.Ln, bias=1e-12)
            nc.scalar.activation(ntl[:], ntl[:], AF.Exp, scale=0.5)
            rntl = e("rntl")
            nc.vector.reciprocal(rntl[:], ntl[:])
            evl = e("evl")
            nc.scalar.activation(evl[:], ntl[:], AF.Exp, scale=2.0)
            nc.vector.tensor_scalar(evl[:], evl[:], 1.0, None, OP.add)
            rq = e("rq")
            nc.vector.reciprocal(rq[:], evl[:])
            tnl = e("tnl")
            nc.vector.tensor_scalar(tnl[:], rq[:], -2.0, 1.0, OP.mult, OP.add)
            nc.vector.tensor_scalar(tnl[:], tnl[:], MAXN, None, OP.min)
            dnl = e("dnl")
            nc.vector.scalar_tensor_tensor(
                dnl[:], rq[:], rq[:], rq[:], OP.mult, OP.subtract)
            Bh = e("Bh")
            nc.vector.tensor_scalar(Bh[:], dnl[:], -4.0, BMIN, OP.mult, OP.max)
            mul_ = e("mul_")
            nc.vector.tensor_tensor(mul_[:], tnl[:], rntl[:], OP.mult)
            kk2l = e("kk2l")
            nc.vector.tensor_scalar(kk2l[:], rntl[:], 2.0 * _A999, 2.0,
                                    OP.mult, OP.min)

            # transpose folded t_last, hyperbolic linear layer
            pT = eps.tile([128, 256], f32, tag="epT")
            nc.tensor.transpose(pT[:, 0:128], tsel[:, 0:128], ident_sb[:])
            nc.tensor.transpose(pT[:, 128:256], tsel[:, 128:256], ident_sb[:])
            tselT = eb.tile([128, 256], f32r, tag="tselT")
            nc.vector.tensor_copy(tselT[:], pT[:])
            wl_sb = []
            for ki in range(8):
                t_ = eb.tile([128, HP1], f32r, tag=f"wl{ki}", name=f"wl{ki}")
                nc.sync.dma_start(t_[:], Wl[ki * 128:(ki + 1) * 128, :])
                wl_sb.append(t_)
            pl = eps.tile([BL, HP1], f32, tag="pl")
            for ki in range(8):
                c0 = (ki % 2) * 128 + (ki // 2) * BL
                for (n0, n1) in ((0, 512), (512, 1024), (1024, 1026)):
                    nc.tensor.matmul(
                        pl[:, n0:n1],
                        tselT[:, c0:c0 + BL],
                        wl_sb[ki][:, n0:n1],
                        start=(ki == 0), stop=(ki == 7))
            scrL = eb.tile([BL, H], f32, tag="scrL")
            s2u = e("es2u")
            nc.scalar.activation(scrL[:], pl[:, 0:H], AF.Square,
                                 accum_out=s2u[:])
            sub = e("esub")
            nc.vector.tensor_copy(sub[:], pl[:, H:H + 1])
            # X = mmatvec(h_last, Wl) folded scalars
            nG = e("enG")
            nc.scalar.activation(nG[:], s2u[:], AF.Ln, bias=1e-12)
            nc.scalar.activation(nG[:], nG[:], AF.Exp, scale=0.5)
            evx = e("evx")
            nc.scalar.activation(evx[:], nG[:], AF.Exp, scale=kk2l[:])
            nc.vector.tensor_scalar(evx[:], evx[:], 1.0, None, OP.add)
            rvx = e("rvx")
            nc.vector.reciprocal(rvx[:], evx[:])
            th = e("eth")
            nc.vector.tensor_scalar(th[:], rvx[:], -2.0, 1.0, OP.mult, OP.add)
            nc.vector.tensor_scalar(th[:], th[:], MAXN, None, OP.min)
            dnx = e("ednx")
            nc.vector.scalar_tensor_tensor(
                dnx[:], rvx[:], rvx[:], rvx[:], OP.mult, OP.subtract)
            B1 = e("eB1")
            nc.vector.tensor_scalar(B1[:], dnx[:], -4.0, BMIN, OP.mult, OP.max)
            x2 = e("ex2")
            nc.vector.tensor_tensor(x2[:], th[:], th[:], OP.mult)
            cwm = e("ecwm")
            nc.vector.reciprocal(cwm[:], nG[:])
            nc.vector.tensor_tensor(cwm[:], cwm[:], th[:], OP.mult)
            # madd(X, bc): y-side is the constant bias point bc
            bc2 = cst_sb[0:BL, C_BC2:C_BC2 + 1]
            Bbc2 = cst_sb[0:BL, C_BBC:C_BBC + 1]
            subt = e("esubt")
            nc.vector.tensor_tensor(subt[:], sub[:], cwm[:], OP.mult)
            A1 = e("eA1")
            nc.vector.tensor_scalar(A1[:], subt[:], 2.0, 1.0, OP.mult, OP.add)
            nc.vector.tensor_scalar(A1[:], A1[:], bc2, None, OP.add)
            ones32e = e("ones32e")
            nc.vector.memset(ones32e[:], 1.0)
            den = e("eden")
            nc.vector.scalar_tensor_tensor(
                den[:], x2[:], bc2, ones32e[:], OP.mult, OP.add)
            nc.vector.scalar_tensor_tensor(
                den[:], subt[:], 2.0, den[:], OP.mult, OP.add)
            aa = e("eaa")
            nc.vector.tensor_tensor(aa[:], A1[:], cwm[:], OP.mult)
            p1 = e("ep1")
            nc.vector.tensor_tensor(p1[:], aa[:], aa[:], OP.mult)
            nc.vector.tensor_tensor(p1[:], p1[:], s2u[:], OP.mult)
            p2 = e("ep2")
            nc.vector.tensor_tensor(p2[:], aa[:], B1[:], OP.mult)
            nc.vector.tensor_tensor(p2[:], p2[:], sub[:], OP.mult)
            nc.vector.tensor_scalar(p2[:], p2[:], 2.0, None, OP.mult)
            p3 = e("ep3")
            nc.vector.tensor_scalar(p3[:], B1[:], B1[:], bc2, OP.mult, OP.mult)
            s2n = e("es2n")
            nc.vector.tensor_tensor(s2n[:], p1[:], p2[:], OP.add)
            nc.vector.tensor_tensor(s2n[:], s2n[:], p3[:], OP.add)
            n1 = e("en1")
            nc.scalar.activation(n1[:], s2n[:], AF.Ln, bias=1e-12)
            nc.scalar.activation(n1[:], n1[:], AF.Exp, scale=0.5)
            rd = e("erd")
            nc.vector.reciprocal(rd[:], den[:])
            gam = e("egam")
            nc.vector.reciprocal(gam[:], n1[:])
            nc.vector.tensor_scalar(gam[:], gam[:], MAXN, None, OP.mult)
            nc.vector.tensor_tensor(gam[:], gam[:], rd[:], OP.min)
            Brepr = e("eBrepr")
            nc.vector.tensor_scalar(Brepr[:], B1[:], Bbc2, rd[:],
                                    OP.mult, OP.mult)
            nc.vector.tensor_scalar(Brepr[:], Brepr[:], BMIN, None, OP.max)
            x2r = e("ex2r")
            nc.vector.tensor_scalar(x2r[:], Brepr[:], -1.0, 1.0,
                                    OP.mult, OP.add)
            c1 = e("ec1")
            nc.vector.tensor_tensor(c1[:], gam[:], A1[:], OP.mult)
            nc.vector.tensor_tensor(c1[:], c1[:], cwm[:], OP.mult)
            c2 = e("ec2")
            nc.vector.tensor_tensor(c2[:], gam[:], B1[:], OP.mult)
            rep = eb.tile([BL, H], f32, tag="rep")
            nc.scalar.activation(rep[:], pl[:, 0:H], AF.Copy, scale=c1[:])
            rep2 = eb.tile([BL, H], f32, tag="rep2")
            nc.vector.scalar_tensor_tensor(
                rep2[:], bcrep_sb[0:BL, :], c2[:], rep[:], OP.mult, OP.add)

            pk = eb.tile([BL, HP2], f32, tag="pk")
            nc.vector.tensor_copy(pk[:, 0:H], rep2[:])
            nc.vector.tensor_copy(pk[:, H:H + 1], x2r[:])
            nc.vector.tensor_copy(pk[:, H + 1:H + 2], Brepr[:])
            nc.sync.dma_start(ccin[:, :], pk[:])
            nc.gpsimd.collective_compute(
                "AllGather", mybir.AluOpType.bypass,
                replica_groups=[list(range(8))],
                ins=[ccin[:, :]], outs=[ccout[:, :]])
            # static full-batch slices: premise rows 0:128, hypothesis 128:256
            ccp = eb.tile([128, HP2], f32, tag="ccp")
            nc.sync.dma_start(ccp[:], ccout[0:128, :])
            cch = eb.tile([128, HP2], f32, tag="cch")
            nc.sync.dma_start(cch[:], ccout[128:256, :])
            prep = ccp[0:128, 0:H]
            hrep = cch[0:128, 0:H]
            px2 = ccp[0:128, H:H + 1]
            hy2c = cch[0:128, H:H + 1]
            pB = ccp[0:128, H + 1:H + 2]
            hB = cch[0:128, H + 1:H + 2]
            ones128 = eb.tile([128, 1], f32, tag="ones128")
            nc.vector.memset(ones128[:], 1.0)

            def f(tag):
                return es.tile([128, 1], f32, tag=tag, name=tag)

            # combine: rep = madd(p_rep, h_rep)
            xyf = f("xyf")
            scrF = eb.tile([128, H], f32, tag="scrF")
            nc.vector.scalar_tensor_tensor(
                scrF[:], prep, 1.0, hrep, OP.mult, OP.mult, accum_out=xyf[:])
            Af = f("Af")
            nc.vector.tensor_scalar(Af[:], xyf[:], 2.0, 1.0, OP.mult, OP.add)
            nc.vector.tensor_scalar(Af[:], Af[:], hy2c, None, OP.add)
            Bf = f("Bf")
            nc.vector.tensor_scalar(Bf[:], px2, -1.0, 1.0, OP.mult, OP.add)
            denf = f("denf")
            nc.vector.scalar_tensor_tensor(
                denf[:], px2, hy2c, ones128[:], OP.mult, OP.add)
            nc.vector.scalar_tensor_tensor(
                denf[:], xyf[:], 2.0, denf[:], OP.mult, OP.add)
            numf = eb.tile([128, H], f32, tag="numf")
            nc.scalar.activation(numf[:], prep, AF.Copy, scale=Af[:])
            numf2 = eb.tile([128, H], f32, tag="numf2")
            nc.vector.scalar_tensor_tensor(
                numf2[:], hrep, Bf[:], numf[:], OP.mult, OP.add)
            s2f = f("s2f")
            nc.scalar.activation(scrF[:], numf2[:], AF.Square, accum_out=s2f[:])
            nf = f("nf")
            nc.scalar.activation(nf[:], s2f[:], AF.Ln, bias=1e-12)
            nc.scalar.activation(nf[:], nf[:], AF.Exp, scale=0.5)
            rdf = f("rdf")
            nc.vector.reciprocal(rdf[:], denf[:])
            gf = f("gf")
            nc.vector.reciprocal(gf[:], nf[:])
            nc.vector.tensor_scalar(gf[:], gf[:], MAXN, None, OP.mult)
            nc.vector.tensor_tensor(gf[:], gf[:], rdf[:], OP.min)
            Brf = f("Brf")
            nc.vector.tensor_scalar(Brf[:], pB, hB, rdf[:], OP.mult, OP.mult)
            nc.vector.tensor_scalar(Brf[:], Brf[:], BMIN, None, OP.max)
            y2f = f("y2f")
            nc.vector.tensor_scalar(y2f[:], Brf[:], -1.0, 1.0, OP.mult, OP.add)
            repf = eb.tile([128, H], f32, tag="repf")
            nc.scalar.activation(repf[:], numf2[:], AF.Copy, scale=gf[:])

            # MLR
            pT2 = eps.tile([128, H], f32, tag="epT2")
            for c in range(8):
                nc.tensor.transpose(
                    pT2[:, c * 128:(c + 1) * 128],
                    repf[:, c * 128:(c + 1) * 128],
                    ident_sb[:])
            repT = eb.tile([128, H], f32r, tag="repT")
            nc.vector.tensor_copy(repT[:], pT2[:])
            mlr_sb = []
            for ki in range(8):
                t_ = eb.tile([128, 2 * NCLS], f32r, tag=f"mlr{ki}",
                             name=f"mlr{ki}")
                nc.sync.dma_start(t_[:], mlrT[ki * 128:(ki + 1) * 128, :])
                mlr_sb.append(t_)
            pm = eps.tile([128, 2 * NCLS], f32, tag="pm")
            for ki in range(8):
                nc.tensor.matmul(
                    pm[:, :],
                    repT[:, ki * 128:(ki + 1) * 128],
                    mlr_sb[ki][:, :],
                    start=(ki == 0), stop=(ki == 7))

            def e3(tag):
                return es.tile([128, NCLS], f32, tag=tag, name=tag)

            xp = e3("xp")
            nc.vector.tensor_copy(xp[:], pm[:, 0:NCLS])
            xa = e3("xa")
            nc.vector.tensor_copy(xa[:], pm[:, NCLS:2 * NCLS])
            pk2 = cst_sb[:, C_PK2:C_PK2 + NCLS]
            pkak = cst_sb[:, C_PKAK:C_PKAK + NCLS]
            anc = cst_sb[:, C_AN:C_AN + NCLS]
            lamc = cst_sb[:, C_LAM:C_LAM + NCLS]
            Bpk = cst_sb[:, C_BPK:C_BPK + NCLS]
            xyk = e3("xyk")
            nc.vector.tensor_scalar(xyk[:], xp[:], -1.0, None, OP.mult)
            Ak = e3("Ak")
            nc.vector.tensor_scalar(Ak[:], xyk[:], 2.0, 1.0, OP.mult, OP.add)
            nc.vector.tensor_scalar(Ak[:], Ak[:], y2f[:], None, OP.add)
            Bk = e3("Bk")
            nc.vector.tensor_scalar(Bk[:], pk2, -1.0, 1.0, OP.mult, OP.add)
            denk = e3("denk")
            nc.vector.tensor_scalar(denk[:], pk2, y2f[:], None, OP.mult)
            nc.vector.tensor_tensor(denk[:], denk[:], xyk[:], OP.add)
            nc.vector.tensor_tensor(denk[:], denk[:], xyk[:], OP.add)
            nc.vector.tensor_scalar(denk[:], denk[:], 1.0, None, OP.add)
            q1k = e3("q1k")
            nc.vector.tensor_tensor(q1k[:], Ak[:], Ak[:], OP.mult)
            nc.vector.tensor_tensor(q1k[:], q1k[:], pk2, OP.mult)
            q2k = e3("q2k")
            nc.vector.tensor_tensor(q2k[:], Ak[:], Bk[:], OP.mult)
            nc.vector.tensor_tensor(q2k[:], q2k[:], xyk[:], OP.mult)
            nc.vector.tensor_scalar(q2k[:], q2k[:], 2.0, None, OP.mult)
            q3k = e3("q3k")
            nc.vector.tensor_tensor(q3k[:], Bk[:], Bk[:], OP.mult)
            nc.vector.tensor_scalar(q3k[:], q3k[:], y2f[:], None, OP.mult)
            s2k = e3("s2k")
            nc.vector.tensor_tensor(s2k[:], q1k[:], q2k[:], OP.add)
            nc.vector.tensor_tensor(s2k[:], s2k[:], q3k[:], OP.add)
            n1k = e3("n1k")
            nc.scalar.activation(n1k[:], s2k[:], AF.Ln, bias=1e-12)
            nc.scalar.activation(n1k[:], n1k[:], AF.Exp, scale=0.5)
            rdk = e3("rdk")
            nc.vector.reciprocal(rdk[:], denk[:])
            gk = e3("gk")
            nc.vector.reciprocal(gk[:], n1k[:])
            nc.vector.tensor_scalar(gk[:], gk[:], MAXN, None, OP.mult)
            nc.vector.tensor_tensor(gk[:], gk[:], rdk[:], OP.min)
            # 1 - |z|^2 via identity: Bz = max(Bpk * Brf * rdk, BMIN)
            Bzk = e3("Bzk")
            nc.vector.tensor_scalar(Bzk[:], Bpk, Brf[:], None, OP.mult)
            nc.vector.tensor_tensor(Bzk[:], Bzk[:], rdk[:], OP.mult)
            nc.vector.tensor_scalar(Bzk[:], Bzk[:], BMIN, None, OP.max)
            zak = e3("zak")
            nc.vector.tensor_tensor(zak[:], Ak[:], pkak, OP.mult)
            nc.vector.tensor_scalar(zak[:], zak[:], -1.0, None, OP.mult)
            q4k = e3("q4k")
            nc.vector.tensor_tensor(q4k[:], Bk[:], xa[:], OP.mult)
            nc.vector.tensor_tensor(zak[:], zak[:], q4k[:], OP.add)
            nc.vector.tensor_tensor(zak[:], zak[:], gk[:], OP.mult)
            vk = e3("vk")
            nc.vector.tensor_tensor(vk[:], Bzk[:], anc, OP.mult)
            nc.vector.reciprocal(vk[:], vk[:])
            nc.vector.tensor_tensor(vk[:], vk[:], zak[:], OP.mult)
            nc.vector.tensor_scalar(vk[:], vk[:], 2.0, None, OP.mult)
            # asinh(v) = sign(v) * ln(|v| + sqrt(v^2+1))  (cancellation-free)
            av = e3("av")
            nc.scalar.activation(av[:], vk[:], AF.Abs)
            sg = e3("sg")
            nc.scalar.activation(sg[:], vk[:], AF.Sign)
            sq = e3("sq")
            nc.vector.tensor_tensor(sq[:], vk[:], vk[:], OP.mult)
            nc.vector.tensor_scalar(sq[:], sq[:], 1.0, None, OP.add)
            nc.scalar.activation(sq[:], sq[:], AF.Ln)
            nc.scalar.activation(sq[:], sq[:], AF.Exp, scale=0.5)
            nc.vector.tensor_tensor(sq[:], sq[:], av[:], OP.add)
            nc.scalar.activation(sq[:], sq[:], AF.Ln)
            nc.vector.tensor_tensor(sq[:], sq[:], sg[:], OP.mult)
            logit = e3("logit")
            nc.vector.tensor_tensor(logit[:], sq[:], anc, OP.mult)
            nc.vector.tensor_tensor(logit[:], logit[:], lamc, OP.mult)
            nc.sync.dma_start(out[:, :], logit[:])

    nc.finalize()
    return nc


def _bf16_pair(x):
    """Split fp32 array into (hi, lo) bf16 pieces: hi = bf16(x),
    lo = bf16(x - hi)."""
    from concourse import mybir
    bd = mybir.dt.np(mybir.dt.bfloat16)
    hi = x.astype(np.float32).astype(bd)
    lo = (x.astype(np.float32) - hi.astype(np.float32)).astype(bd)
    return hi, lo


def _host_prep(inputs, s_steps=S):
    f = np.float32
    emb = inputs["emb_table"].astype(f)
    in_maps = []
    for c in range(8):
        q, r = c % 4, c // 4
        ids = (inputs["premise"] if r == 0 else inputs["hypothesis"])[
            q * BL:(q + 1) * BL, :s_steps].astype(np.int64)
        lens = (inputs["p_len"] if r == 0 else inputs["h_len"])[
            q * BL:(q + 1) * BL].astype(np.int64)
        lens = np.minimum(lens, s_steps)
        U = (inputs["Up"] if r == 0 else inputs["Uh"]).astype(f)
        W = (inputs["Wp"] if r == 0 else inputs["Wh"]).astype(f)
        bv = (inputs["bp"] if r == 0 else inputs["bh"]).astype(f)
        Wlin = (inputs["Wcp"] if r == 0 else inputs["Wch"]).astype(f)
        bc = (inputs["bcp"] if r == 0 else inputs["bch"]).astype(f)
        a_mlr = inputs["a_mlr"].astype(f)
        p_mlr = inputs["p_mlr"].astype(f)

        x = emb[ids]
        xTv = np.ascontiguousarray(x.transpose(2, 1, 0).reshape(E, s_steps * BL))
        xTh_v, xTl_v = _bf16_pair(xTv)
        xn2 = np.maximum((x.astype(np.float64) ** 2).sum(-1), 1e-12)
        xn2v = np.ascontiguousarray(
            xn2.transpose(1, 0).reshape(-1, 128).T).astype(f)
        zc = np.zeros((H, 1), f)
        zc2 = np.zeros((H, 2), f)
        Ucv = np.concatenate([U, (U @ bv)[:, None], np.zeros((E, 1), f)], 1)
        # W packed per col-group g: [W[:,256g:256g+256] | bcol | pad] where
        # bcol = W@b only in group 3, zeros elsewhere.
        Wb = (W @ bv)[:, None]
        Wrv = np.concatenate(
            [W[:, 0:256], zc2, W[:, 256:512], zc2, W[:, 512:768], zc2,
             W[:, 768:1024], Wb, zc], 1)
        Wlv = np.concatenate([Wlin, (Wlin @ bc)[:, None], zc], 1)
        Uch_v, Ucl_v = _bf16_pair(Ucv)
        Wrh_v, Wrl_v = _bf16_pair(Wrv)
        b2 = float(bv.astype(np.float64) @ bv.astype(np.float64))
        bc2 = float(bc.astype(np.float64) @ bc.astype(np.float64))
        p2 = np.sum(p_mlr.astype(np.float64) ** 2, -1)
        cstv = np.zeros((128, CSTW), f)
        cstv[:, C_OPB2] = 1.0 + b2
        cstv[:, C_NB2] = -b2
        cstv[:, C_B2] = b2
        cstv[:, C_BB] = 1.0 - b2
        cstv[:, C_BC2] = bc2
        cstv[:, C_BBC] = 1.0 - bc2
        cstv[:, C_PK2:C_PK2 + NCLS] = p2
        cstv[:, C_PKAK:C_PKAK + NCLS] = np.sum(
            p_mlr.astype(np.float64) * a_mlr.astype(np.float64), -1)
        cstv[:, C_AN:C_AN + NCLS] = np.sqrt(
            np.maximum(np.sum(a_mlr.astype(np.float64) ** 2, -1), 1e-12))
        cstv[:, C_LAM:C_LAM + NCLS] = 2.0 / np.maximum(1.0 - p2, 1e-5)
        cstv[:, C_BPK:C_BPK + NCLS] = 1.0 - p2
        eqv = (np.arange(s_steps)[None, :] == (lens - 1)[:, None]).astype(f)
        brep_f = np.tile(bv.reshape(4, 256)[:, None, :],
                         (1, BL, 1)).reshape(128, 256).astype(f)
        in_maps.append({
            "xTh": xTh_v, "xTl": xTl_v, "xn2d": xn2v,
            "Uch": Uch_v, "Ucl": Ucl_v, "Wrh": Wrh_v, "Wrl": Wrl_v,
            "Wl": Wlv,
            "brep": brep_f,
            "bcrep": np.broadcast_to(bc, (128, H)).copy(),
            "mlrT": np.concatenate([p_mlr.T, a_mlr.T], 1).astype(f),
            "cst": cstv, "eqm": np.tile(eqv, (4, 1)).astype(f),
            "ident": np.eye(128, dtype=f),
            "i4m": np.tile(np.eye(BL, dtype=f), (4, 4)),
        })
    return in_maps


def kernel(premise, p_len, hypothesis, h_len, emb_table, Wp, Up, bp,
           Wh, Uh, bh, Wcp, bcp, Wch, bch, a_mlr, p_mlr,
           s_steps=S, trace=False):
    from concourse.bass_utils import run_bass_kernel_spmd
    inputs = dict(premise=premise, p_len=p_len, hypothesis=hypothesis,
                  h_len=h_len, emb_table=emb_table, Wp=Wp, Up=Up, bp=bp,
                  Wh=Wh, Uh=Uh, bh=bh, Wcp=Wcp, bcp=bcp, Wch=Wch, bch=bch,
                  a_mlr=a_mlr, p_mlr=p_mlr)
    inputs = {k: np.asarray(v) for k, v in inputs.items()}
    if s_steps not in _CACHE:
        _CACHE[s_steps] = _build(s_steps)
    nc = _CACHE[s_steps]
    in_maps = _host_prep(inputs, s_steps)
    res = run_bass_kernel_spmd(nc, in_maps, core_ids=list(range(8)),
                               trace=trace)
    kernel.last_results = res
    return res.results[0]["out"].astype(np.float32)


kernel.last_results = None



# revision 42
# speedup vs baseline: 1.1797x; 1.1797x over previous
"""Trainium2 Bass kernel for the hyperbolic (Poincare-ball) AddRNN NLI model.

Sharding: 8 cores SPMD. Core c: RNN role r=c//4 (0=premise, 1=hypothesis),
batch quarter q=c%4 (rows 32q..32q+31). Each core runs the full sequential
scan for its (role, quarter).

Architecture (beyond the original folded-scalar implementation):
- Folded-H layout: every wide [32,1024] tensor lives as [128,256], with
  partition 32j+b holding batch row b's H-chunk j. All wide DVE/ACT ops
  (square-accums, state update, tanh, masked accumulates) run 4x faster
  than batch-major, and the per-step state transpose is 2 full 128x128 PE
  transposes instead of 8 thin ones.
- The recurrent W-matmul runs as 4 concurrent PE column-groups
  (tile_position=(0,32j)), group j computing H cols [256j,256j+258) of
  t@W straight into its folded PSUM strip; 96 bf16 N=258 matmuls/step at
  ~110ns per 4-way group when warm. bf16 hi/lo 3-pass GEMM (th@Wh + th@Wl
  + tl@Wh) keeps per-dot error ~2^-18 (fp32r single-pass amplifies to
  ~2e-2 over 256 steps - too close to the gate). The W@b bias-dot rides
  as column 256 of group 3's weight tile.
- Cross-strip reductions (|uG|^2, <uG,y>, <uG,Wb>, |t|^2) go through tiny
  fp32 matmuls with I4rep = tile(I32,(4,4)) which both sum the 4 strips
  AND replicate to all 128 partitions, so the whole Mobius scalar chain
  runs on [128,1] tiles and its outputs feed the folded state update
  directly (no broadcast). Three decoupled partial->reduce paths let the
  chain's first Ln wait only on the |uG|^2 path.
- P1 (per-token y = mmatvec(x,U) GEMM) is interleaved into the scan: one
  128-token tile per 2 steps plus chunked table-chains, hiding ~0.45ms of
  prologue inside the scan's serial-latency gaps (Tile tracks the
  ydr DRAM RAW deps). P1 writes ydr pre-folded via a 3-level DMA AP.
- HAM keep-warm: 3 bursts of 3 fp32 dummy matmuls, data-dependent on
  mid-chain scalars, keep the PE activity monitor at K=8/8 through the
  ~9us scalar-chain window; without them the whole matmul phase runs at
  1.2GHz (measured ~5.3us vs ~2.9us warm).
- ACT tables pinned to natural_log_exp_and_others + sigmoid_and_others
  (2 hidden table loads/step). Cancellation-free gyro algebra:
  1-|x(+)y|^2 = (1-|x|^2)(1-|y|^2)/den through both madds;
  1-tanh^2 = 4rv(1-rv); den1 = A1 - y2*B1 (uses 1-th^2 == B1 exactly,
  clamps included); den2 = A2 - b2*B2; fused tensor_scalar/stt forms
  throughout; artanh via Ln(n2, bias=1) - Ln(Bz2).
- Late-chain shortcut: the wide update W2 = uG + (cycp/cu)y + (cb/cu)b is
  built MID-chain (the ratios cycp/cu = B1*cyt/aa and cb/cu =
  B2/(A2*gam*aa) need no late chain values), and cu is applied through
  the tanh's free scale slot - removing cycp->w0->w1->w2 (~1.6us) from
  the serial tail. P1 tiles are injected at the END of the step body so
  their DVE copies queue into the next chain window, not ahead of the
  reduce ops; w0's old ACT Copy is gone so the tanh table-load no longer
  stalls TANH in the ACT FIFO.
Per-step ~16.2us: matmul ~2.8 + reduce ~1.3 + scalar chain ~9 +
tanh/transpose ~2. History: 8.36ms baseline -> 5.88 (folding +
col-tiling) -> 5.25 (replicated chain, P1 interleave, fusions) -> 4.78
(HAM keep-warm bursts) -> 4.55 (cu-scale tail cut) -> 4.54ms (early lnB
so the tanh table-load starts sooner; cut = L*g1*aa drops the m2 op).
Tried+reverted: epilogue weight preload into cpool (+3.7us/step SBUF
shuffle), split CAST per transpose block (+0.75us/step).
"""
import numpy as np

B, S, E, H, V, NCLS = 128, 256, 300, 1024, 32000, 3
BL = 32
MAXN = 1.0 - 1e-3
BMIN = float(1.0 - np.float64(MAXN) * np.float64(MAXN))  # 1-MAXN^2
_A999 = float(np.arctanh(np.float64(MAXN)))

_CACHE = {}

# cst column map
C_B2, C_BB, C_BC2, C_BBC = 0, 1, 2, 3
C_PK2, C_PKAK, C_AN, C_LAM, C_BPK = 4, 7, 10, 13, 16
C_OPB2, C_NB2 = 19, 20
CSTW = 21


def _make_bacc():
    """Bacc with activation-table insertion pinned to two sets so the
    Ln/Exp/Square/Copy cluster and the wide Tanh never thrash tables."""
    from concourse import bacc
    from concourse.hw_specs import get_activation_tables
    import bass_rust as _bass_rust

    class _Bacc2(bacc.Bacc):
        def insert_act_table_loads(self):
            from concourse import mybir as mb
            has_activation = any(
                isinstance(i, mb.InstActivation)
                for b in self.main_func.blocks
                for i in b.instructions
            )
            if not has_activation:
                return
            # Keep the canonical list (act_func_set_id is positional into
            # act_info.json) but empty out every set except the two we pin,
            # so the chooser can only bind activations to those.
            tabs = get_activation_tables(self.m.arch)
            keep = ("natural_log_exp_and_others", "sigmoid_and_others")
            tables = [(k, v if k in keep else set()) for k, v in tabs.items()]
            _bass_rust.insert_act_table_loads(self, tables)

    return _Bacc2()


def _build(s_steps, use_gpsimd_sel=None):
    import os
    if use_gpsimd_sel is None:
        use_gpsimd_sel = os.environ.get("V2_GPSEL", "1") == "1"
    use_f32r = os.environ.get("V2_F32R", "1") == "1"
    use_tabs = os.environ.get("V2_TABS", "1") == "1"

    import concourse.tile as tile
    from concourse import mybir

    f32 = mybir.dt.float32
    f32r = mybir.dt.float32r if use_f32r else mybir.dt.float32
    bf16 = mybir.dt.bfloat16
    AF = mybir.ActivationFunctionType
    OP = mybir.AluOpType

    NT = BL * s_steps
    NTILES = NT // 128
    HP1 = H + 2  # weights padded to even width for fp32r ISA rules
    HP2 = H + 2  # allgather payload: rep | x2 | B

    if use_tabs:
        nc = _make_bacc()
    else:
        from concourse import bacc as _bacc
        nc = _bacc.Bacc()

    # register the Ln-bias constant as a const AP (only 0.0/1.0 are built in)
    _cb = nc.alloc_sbuf_tensor("const-f32-1em12", [128, 1], f32)
    nc.gpsimd.memset(_cb.ap(), 1e-12)
    nc.const_aps.aps[(f32, 1e-12)] = _cb.ap()
    nc.all_engine_barrier()

    GW = 258          # per-col-group W width: 256 H cols + b col + pad
    xTh = nc.declare_dram_parameter("xTh", [E, NT], bf16, isOutput=False)
    xTl = nc.declare_dram_parameter("xTl", [E, NT], bf16, isOutput=False)
    xn2d = nc.declare_dram_parameter("xn2d", [128, NTILES], f32, isOutput=False)
    Uch = nc.declare_dram_parameter("Uch", [E, HP1], bf16, isOutput=False)
    Ucl = nc.declare_dram_parameter("Ucl", [E, HP1], bf16, isOutput=False)
    Wrh = nc.declare_dram_parameter("Wrh", [H, 4 * GW], bf16, isOutput=False)
    Wrl = nc.declare_dram_parameter("Wrl", [H, 4 * GW], bf16, isOutput=False)
    Wl = nc.declare_dram_parameter("Wl", [H, HP1], f32r, isOutput=False)
    brep = nc.declare_dram_parameter("brep", [128, 256], f32, isOutput=False)
    bcrep = nc.declare_dram_parameter("bcrep", [128, H], f32, isOutput=False)
    mlrT = nc.declare_dram_parameter("mlrT", [H, 2 * NCLS], f32r, isOutput=False)
    cst = nc.declare_dram_parameter("cst", [128, CSTW], f32, isOutput=False)
    eqm = nc.declare_dram_parameter("eqm", [128, s_steps], f32, isOutput=False)
    ident = nc.declare_dram_parameter("ident", [128, 128], f32, isOutput=False)
    i4m = nc.declare_dram_parameter("i4m", [128, 128], f32, isOutput=False)
    out = nc.declare_dram_parameter("out", [B, NCLS], f32, isOutput=True)

    ydr = nc.dram_tensor("ydr", [s_steps * 128, 256], f32)
    ccin = nc.dram_tensor("ccin", [BL, HP2], f32)
    ccout = nc.dram_tensor("ccout", [8 * BL, HP2], f32, addr_space="Shared")

    with tile.TileContext(nc) as tc:
      with tc.tile_pool(name="const", bufs=1) as cpool:
        # ---------------- constants to SBUF --------------------------------
        wh_sb, wl_sb = [], []
        for ki in range(8):
            th_ = cpool.tile([128, 4 * GW], bf16, tag=f"wh{ki}", name=f"wh{ki}")
            nc.sync.dma_start(th_[:], Wrh[ki * 128:(ki + 1) * 128, :])
            wh_sb.append(th_)
            tl_ = cpool.tile([128, 4 * GW], bf16, tag=f"wl{ki}b", name=f"wl{ki}b")
            nc.sync.dma_start(tl_[:], Wrl[ki * 128:(ki + 1) * 128, :])
            wl_sb.append(tl_)
        uh_sb, ul_sb = [], []
        for c, kc in enumerate((128, 128, E - 256)):
            th_ = cpool.tile([128, HP1], bf16, tag=f"uh{c}", name=f"uh{c}")
            nc.sync.dma_start(th_[0:kc, :], Uch[c * 128:c * 128 + kc, :])
            uh_sb.append(th_)
            tl_ = cpool.tile([128, HP1], bf16, tag=f"ul{c}", name=f"ul{c}")
            nc.sync.dma_start(tl_[0:kc, :], Ucl[c * 128:c * 128 + kc, :])
            ul_sb.append(tl_)
        brep_sb = cpool.tile([128, 256], f32, tag="brep")
        nc.sync.dma_start(brep_sb[:], brep[:, :])
        bcrep_sb = cpool.tile([128, H], f32, tag="bcrep")
        nc.sync.dma_start(bcrep_sb[:], bcrep[:, :])
        cst_sb = cpool.tile([128, CSTW], f32, tag="cst")
        nc.sync.dma_start(cst_sb[:], cst[:, :])
        ident_sb = cpool.tile([128, 128], f32, tag="ident")
        nc.sync.dma_start(ident_sb[:], ident[:, :])
        i4_sb = cpool.tile([128, 128], f32, tag="i4m")
        nc.sync.dma_start(i4_sb[:], i4m[:, :])
        eqm_sb = cpool.tile([128, s_steps], f32, tag="eqm")
        nc.sync.dma_start(eqm_sb[:], eqm[:, :])
        xn2a = cpool.tile([128, NTILES], f32, tag="xn2a")
        nc.sync.dma_start(xn2a[:], xn2d[:, :])
        un2a = cpool.tile([128, NTILES], f32, tag="un2a")
        uba = cpool.tile([128, NTILES], f32, tag="uba")
        # per-step tables, replicated x4 along partitions: [128, s_steps]
        y2s = cpool.tile([128, s_steps], f32, tag="y2s")
        y2p1s = cpool.tile([128, s_steps], f32, tag="y2p1s")
        Bys = cpool.tile([128, s_steps], f32, tag="Bys")
        cys = cpool.tile([128, s_steps], f32, tag="cys")
        sybs = cpool.tile([128, s_steps], f32, tag="sybs")
        ny2s = cpool.tile([128, s_steps], f32, tag="ny2s")

        b2c = cst_sb[:, C_B2:C_B2 + 1]
        Bbc = cst_sb[:, C_BB:C_BB + 1]
        opb2c = cst_sb[:, C_OPB2:C_OPB2 + 1]
        nb2c = cst_sb[:, C_NB2:C_NB2 + 1]

        # =============== P1: prologue  y = mmatvec(x, U) ====================
        # P1 (prologue GEMM y = mmatvec(x,U)) and P2 (the scan) share one
        # pool scope: P1 tiles 8..NTILES-1 and the chunked table-chains are
        # emitted INSIDE the step loop (one tile per 2 steps), so their
        # PE/DVE/ACT work hides in the scan's serial-latency gaps. DRAM RAW
        # deps (ydr write -> yt read) are tracked by Tile, order is safe.
        with (
            tc.tile_pool(name="bigio", bufs=2) as bpool,
            tc.tile_pool(name="psA", bufs=1, space="PSUM") as psA,
            tc.tile_pool(name="scal", bufs=2) as spool,
            tc.tile_pool(name="state", bufs=1) as stp,
            tc.tile_pool(name="rbig", bufs=2) as rb,
            tc.tile_pool(name="rps", bufs=2, space="PSUM") as rps,
            tc.tile_pool(name="rpsT", bufs=1, space="PSUM") as rpsT,
            tc.tile_pool(name="rpsS", bufs=1, space="PSUM") as rpsS,
            tc.tile_pool(name="rsc", bufs=3) as rs,
        ):
            CH = ((0, 512), (512, 1024), (1024, 1026))

            def p1_tile(j):
                xkh = bpool.tile([128, 3 * 128], bf16, tag="xkh")
                nc.sync.dma_start(
                    xkh[:, 0:256].rearrange("p (c m) -> p c m", c=2),
                    xTh[0:256, j * 128:(j + 1) * 128].rearrange(
                        "(c p) m -> p c m", c=2))
                nc.sync.dma_start(
                    xkh[0:E - 256, 256:384],
                    xTh[256:E, j * 128:(j + 1) * 128])
                xkl = bpool.tile([128, 3 * 128], bf16, tag="xkl")
                nc.sync.dma_start(
                    xkl[:, 0:256].rearrange("p (c m) -> p c m", c=2),
                    xTl[0:256, j * 128:(j + 1) * 128].rearrange(
                        "(c p) m -> p c m", c=2))
                nc.sync.dma_start(
                    xkl[0:E - 256, 256:384],
                    xTl[256:E, j * 128:(j + 1) * 128])

                pu = psA.tile([128, HP1], f32, tag="pu")
                for c, kc in enumerate((128, 128, E - 256)):
                    for (n0, n1) in CH:
                        nc.tensor.matmul(
                            pu[:, n0:n1],
                            xkh[0:kc, c * 128:c * 128 + 128],
                            uh_sb[c][0:kc, n0:n1],
                            start=(c == 0), stop=False)
                    for (n0, n1) in CH:
                        nc.tensor.matmul(
                            pu[:, n0:n1],
                            xkh[0:kc, c * 128:c * 128 + 128],
                            ul_sb[c][0:kc, n0:n1],
                            start=False, stop=False)
                for c, kc in enumerate((128, 128, E - 256)):
                    for (n0, n1) in CH:
                        nc.tensor.matmul(
                            pu[:, n0:n1],
                            xkl[0:kc, c * 128:c * 128 + 128],
                            uh_sb[c][0:kc, n0:n1],
                            start=False, stop=(c == 2))
                ysc = bpool.tile([128, H], f32, tag="ysc")
                nc.vector.tensor_copy(ysc[:], pu[:, 0:H])
                nc.vector.tensor_copy(uba[:, j:j + 1], pu[:, H:H + 1])
                # folded scatter: ydr[(4j+sl)*128 + jj*32 + b, c] =
                #   y[b, 4j+sl][256*jj + c]
                for sl in range(4):
                    nc.sync.dma_start(
                        ydr[(j * 4 + sl) * 128:(j * 4 + sl + 1) * 128,
                            :].rearrange("(jj b) c -> b jj c", jj=4, b=BL),
                        ysc[sl * BL:(sl + 1) * BL, :].rearrange(
                            "b (jj c) -> b jj c", jj=4))
                scrH = bpool.tile([128, H], f32, tag="scrH")
                nc.scalar.activation(scrH[:], ysc[:], AF.Square,
                                     accum_out=un2a[:, j:j + 1])

            def p1_chain(c0, c1):
                # chunked prologue scalar chain on [128, c1-c0]
                W_ = c1 - c0

                def sc(tag):
                    return spool.tile([128, W_], f32, tag=f"{tag}_{W_}",
                                      name=f"{tag}_{c0}")

                xn = sc("p_xn")
                nc.scalar.activation(xn[:], xn2a[:, c0:c1], AF.Ln, bias=1e-12)
                nc.scalar.activation(xn[:], xn[:], AF.Exp, scale=0.5)
                rxn = sc("p_rxn")
                nc.vector.reciprocal(rxn[:], xn[:])
                pxa = sc("p_pxa")
                nc.vector.tensor_scalar(pxa[:], xn[:], 1.0, None, OP.add)
                mxa = sc("p_mxa")
                nc.vector.tensor_scalar(mxa[:], xn[:], -1.0, 1.0,
                                        OP.mult, OP.add)
                nc.vector.reciprocal(mxa[:], mxa[:])
                nc.vector.tensor_tensor(pxa[:], pxa[:], mxa[:], OP.mult)
                lnr = sc("p_lnr")
                nc.scalar.activation(lnr[:], pxa[:], AF.Ln)  # = 2*artanh(xn)
                un = sc("p_un")
                nc.scalar.activation(un[:], un2a[:, c0:c1], AF.Ln, bias=1e-12)
                nc.scalar.activation(un[:], un[:], AF.Exp, scale=0.5)
                run = sc("p_run")
                nc.vector.reciprocal(run[:], un[:])
                arg = sc("p_arg")
                nc.vector.tensor_tensor(arg[:], un[:], rxn[:], OP.mult)
                nc.vector.tensor_tensor(arg[:], arg[:], lnr[:], OP.mult)
                # arg = 2*(un/xn)*artanh(xn); tanh = 1-2/(e^arg+1)
                ev = sc("p_ev")
                nc.scalar.activation(ev[:], arg[:], AF.Exp)
                nc.vector.tensor_scalar(ev[:], ev[:], 1.0, None, OP.add)
                rv = sc("p_rv")
                nc.vector.reciprocal(rv[:], ev[:])
                th = sc("p_th")
                nc.vector.tensor_scalar(th[:], rv[:], -2.0, 1.0,
                                        OP.mult, OP.add)
                nc.vector.tensor_scalar(th[:], th[:], MAXN, None, OP.min)
                y2a = sc("p_y2a")
                nc.vector.tensor_tensor(y2a[:], th[:], th[:], OP.mult)
                y2p1a = sc("p_y2p1a")
                nc.vector.tensor_scalar(y2p1a[:], y2a[:], 1.0, None, OP.add)
                dneg = sc("p_dneg")
                nc.vector.tensor_tensor(dneg[:], rv[:], rv[:], OP.mult)
                nc.vector.tensor_tensor(dneg[:], dneg[:], rv[:], OP.subtract)
                Bya = sc("p_Bya")
                nc.vector.tensor_scalar(Bya[:], dneg[:], -4.0, BMIN,
                                        OP.mult, OP.max)
                cya = sc("p_cya")
                nc.vector.tensor_tensor(cya[:], th[:], run[:], OP.mult)
                syba = sc("p_syba")
                nc.vector.tensor_tensor(syba[:], cya[:], uba[:, c0:c1],
                                        OP.mult)
                ny2a = sc("p_ny2a")
                nc.vector.tensor_scalar(ny2a[:], y2a[:], -1.0, None, OP.mult)
                for g in range(4):
                    for jj in range(4):
                        dst = slice(jj * BL, (jj + 1) * BL)
                        sg = slice(g * BL, (g + 1) * BL)
                        cs = slice(4 * c0 + g, 4 * c1, 4)
                        nc.sync.dma_start(y2s[dst, cs], y2a[sg, :])
                        nc.sync.dma_start(y2p1s[dst, cs], y2p1a[sg, :])
                        nc.sync.dma_start(Bys[dst, cs], Bya[sg, :])
                        nc.sync.dma_start(cys[dst, cs], cya[sg, :])
                        nc.sync.dma_start(sybs[dst, cs], syba[sg, :])
                        nc.sync.dma_start(ny2s[dst, cs], ny2a[sg, :])

            # prefix: first 8 tiles + their tables (feeds steps 0..31)
            PRE = min(8, NTILES)
            for j in range(PRE):
                p1_tile(j)
            p1_chain(0, PRE)

            tTh = stp.tile([128, 256], bf16, tag="tTh")
            nc.vector.memset(tTh[:].bitcast(mybir.dt.uint16), 0)
            tTl = stp.tile([128, 256], bf16, tag="tTl")
            nc.vector.memset(tTl[:].bitcast(mybir.dt.uint16), 0)
            kk2 = stp.tile([128, 1], f32, tag="kk2")
            nc.vector.memset(kk2[:], 2.0)
            tsel = cpool.tile([128, 256], f32, tag="tsel", name="tsel")
            nc.vector.memset(tsel[:], 0.0)
            s2tsel = cpool.tile([BL, 1], f32, tag="s2tsel", name="s2tsel")
            nc.vector.memset(s2tsel[:], 0.0)

            def stat(tt, ki):
                c0 = (ki % 2) * 128 + (ki // 2) * BL
                return tt[:, c0:c0 + BL]

            def s(tag):
                return rs.tile([128, 1], f32, tag=tag, name=tag)

            next_tile = PRE
            next_chain = PRE
            for t in range(s_steps):
                yt = rb.tile([128, 256], f32, tag="yt")
                nc.sync.dma_start(yt[:], ydr[t * 128:(t + 1) * 128, :])
                pT = rpsT.tile([128, 256], f32, tag="pT")

                # --- W matmul: 4 col-groups concurrent, k-outer -----------
                pu = rps.tile([128, GW], f32, tag="rpu")
                for ki in range(8):
                    sh = stat(tTh, ki)
                    for g in range(4):
                        nc.tensor.matmul(
                            pu[g * BL:(g + 1) * BL, :], sh,
                            wh_sb[ki][:, g * GW:(g + 1) * GW],
                            start=(ki == 0), stop=False,
                            tile_position=(0, g * BL))
                        nc.tensor.matmul(
                            pu[g * BL:(g + 1) * BL, :], sh,
                            wl_sb[ki][:, g * GW:(g + 1) * GW],
                            start=False, stop=False,
                            tile_position=(0, g * BL))
                for ki in range(8):
                    sl_ = stat(tTl, ki)
                    for g in range(4):
                        nc.tensor.matmul(
                            pu[g * BL:(g + 1) * BL, :], sl_,
                            wh_sb[ki][:, g * GW:(g + 1) * GW],
                            start=False, stop=(ki == 7),
                            tile_position=(0, g * BL))

                y2 = y2s[:, t:t + 1]
                ny2 = ny2s[:, t:t + 1]
                y2p1 = y2p1s[:, t:t + 1]
                By = Bys[:, t:t + 1]
                cyt = cys[:, t:t + 1]
                syb = sybs[:, t:t + 1]

                # --- folded norms: three decoupled partial->reduce paths --
                scr = rb.tile([128, 256], f32, tag="scr")
                prtA = rb.tile([128, 1], f32, tag="prtA")
                nc.scalar.activation(scr[:], pu[:, 0:256], AF.Square,
                                     accum_out=prtA[:])
                prtC = rb.tile([128, 1], f32, tag="prtC")
                nc.vector.tensor_copy(prtC[:], pu[:, 256:257])
                scr2 = rb.tile([128, 256], f32, tag="scr2")
                prtB = rb.tile([128, 1], f32, tag="prtB")
                nc.vector.scalar_tensor_tensor(
                    scr2[:], pu[:, 0:256], 1.0, yt[:],
                    OP.mult, OP.mult, accum_out=prtB[:])
                psmA = rpsS.tile([128, 1], f32, tag="psmA")
                psmX = rpsS.tile([128, 3], f32, tag="psmX")
                nc.tensor.matmul(psmA[:], i4_sb[:], prtA[:],
                                 start=True, stop=True)
                nc.tensor.matmul(psmX[:, 1:2], i4_sb[:], prtC[:],
                                 start=True, stop=True)
                nc.tensor.matmul(psmX[:, 0:1], i4_sb[:], prtB[:],
                                 start=True, stop=True)
                s2u = psmA[:]
                suy = psmX[:, 0:1]
                sub = psmX[:, 1:2]

                # --- X-norm block -----------------------------------------
                lnu = s("lnu")
                nc.scalar.activation(lnu[:], s2u, AF.Ln, bias=1e-12)
                nG = s("nG")
                nc.scalar.activation(nG[:], lnu[:], AF.Exp, scale=0.5)
                ev = s("ev")
                nc.scalar.activation(ev[:], nG[:], AF.Exp, scale=kk2[:])
                rnG = s("rnG")
                nc.scalar.activation(rnG[:], lnu[:], AF.Exp, scale=-0.5)
                q = s("q")
                nc.vector.tensor_scalar(q[:], ev[:], 1.0, None, OP.add)
                rv = s("rv")
                nc.vector.reciprocal(rv[:], q[:])
                th = s("th")
                nc.vector.tensor_scalar(th[:], rv[:], -2.0, 1.0, OP.mult, OP.add)
                nc.vector.tensor_scalar(th[:], th[:], MAXN, None, OP.min)
                dneg = s("dneg")
                nc.vector.scalar_tensor_tensor(
                    dneg[:], rv[:], rv[:], rv[:], OP.mult, OP.subtract)
                B1 = s("B1")
                nc.vector.tensor_scalar(B1[:], dneg[:], -4.0, BMIN,
                                        OP.mult, OP.max)
                cwm = s("cwm")
                nc.vector.tensor_tensor(cwm[:], th[:], rnG[:], OP.mult)
                for db in range(3):
                    nc.tensor.matmul(pT[0:1, 0:256], cwm[0:128, 0:1],
                                     brep_sb[:, 0:256],
                                     start=True, stop=True,
                                     tile_position=(0, 0),
                                     skip_group_check=True)

                # --- madd1 scalars ----------------------------------------
                xy1 = s("xy1")
                nc.vector.tensor_scalar(xy1[:], suy, cyt, cwm[:],
                                        OP.mult, OP.mult)
                r1 = s("r1")
                nc.vector.tensor_scalar(r1[:], suy, cyt, B1[:],
                                        OP.mult, OP.mult)
                A1 = s("A1")
                nc.vector.tensor_scalar(A1[:], xy1[:], 2.0, y2p1,
                                        OP.mult, OP.add)
                den1 = s("den1")
                nc.vector.tensor_scalar(den1[:], B1[:], ny2, A1[:],
                                        OP.mult, OP.add)
                aa = s("aa")
                nc.vector.tensor_tensor(aa[:], A1[:], cwm[:], OP.mult)
                raa = s("raa")
                nc.vector.reciprocal(raa[:], aa[:])
                r1c = s("r1c")
                nc.vector.tensor_scalar(r1c[:], B1[:], cyt, raa[:],
                                        OP.mult, OP.mult)
                r2 = s("r2")
                nc.vector.tensor_scalar(r2[:], aa[:], s2u, None, OP.mult)
                r3 = s("r3")
                nc.vector.scalar_tensor_tensor(
                    r3[:], r1[:], 2.0, r2[:], OP.mult, OP.add)
                r5 = s("r5")
                nc.vector.tensor_scalar(r5[:], B1[:], B1[:], y2,
                                        OP.mult, OP.mult)
                s2n1 = s("s2n1")
                nc.vector.scalar_tensor_tensor(
                    s2n1[:], r3[:], aa[:], r5[:], OP.mult, OP.add)
                rn1 = s("rn1")
                nc.scalar.activation(rn1[:], s2n1[:], AF.Ln, bias=1e-12)
                nc.scalar.activation(rn1[:], rn1[:], AF.Exp, scale=-0.5)
                rd1 = s("rd1")
                nc.vector.reciprocal(rd1[:], den1[:])
                gam = s("gam")
                nc.vector.tensor_scalar(gam[:], rn1[:], MAXN, rd1[:],
                                        OP.mult, OP.min)
                for db in range(3):
                    nc.tensor.matmul(pT[0:1, 0:256], gam[0:128, 0:1],
                                     brep_sb[:, 0:256],
                                     start=True, stop=True,
                                     tile_position=(0, 0),
                                     skip_group_check=True)
                B2 = s("B2")
                nc.vector.tensor_scalar(B2[:], B1[:], By, rd1[:],
                                        OP.mult, OP.mult)
                nc.vector.tensor_scalar(B2[:], B2[:], BMIN, None, OP.max)
                x2b = s("x2b")
                nc.vector.tensor_scalar(x2b[:], B2[:], -1.0, 1.0,
                                        OP.mult, OP.add)

                # --- madd2 scalars (with hyperbolic bias b) ---------------
                t3 = s("t3")
                nc.vector.tensor_scalar(t3[:], aa[:], sub, None, OP.mult)
                t5 = s("t5")
                nc.vector.scalar_tensor_tensor(
                    t5[:], B1[:], syb, t3[:], OP.mult, OP.add)
                xy2d = s("xy2d")
                nc.vector.tensor_scalar(xy2d[:], t5[:], gam[:], 2.0,
                                        OP.mult, OP.mult)
                A2 = s("A2")
                nc.vector.tensor_scalar(A2[:], xy2d[:], opb2c, None, OP.add)
                den2 = s("den2")
                nc.vector.scalar_tensor_tensor(
                    den2[:], B2[:], nb2c, A2[:], OP.mult, OP.add)
                g1 = s("g1")
                nc.vector.tensor_scalar(g1[:], A2[:], gam[:], None, OP.mult)
                rg1 = s("rg1")
                nc.vector.reciprocal(rg1[:], g1[:])
                r2c = s("r2c")
                nc.vector.tensor_scalar(r2c[:], B2[:], raa[:], rg1[:],
                                        OP.mult, OP.mult)
                W1 = rb.tile([128, 256], f32, tag="W1")
                nc.vector.scalar_tensor_tensor(
                    W1[:], yt[:], r1c[:], pu[:, 0:256], OP.mult, OP.add)
                W2 = rb.tile([128, 256], f32, tag="W2")
                nc.vector.scalar_tensor_tensor(
                    W2[:], brep_sb[:], r2c[:], W1[:], OP.mult, OP.add)
                u1 = s("u1")
                nc.vector.tensor_tensor(u1[:], A2[:], x2b[:], OP.mult)
                u3 = s("u3")
                nc.vector.scalar_tensor_tensor(
                    u3[:], xy2d[:], B2[:], u1[:], OP.mult, OP.add)
                u5 = s("u5")
                nc.vector.tensor_scalar(u5[:], B2[:], B2[:], b2c,
                                        OP.mult, OP.mult)
                s2n2 = s("s2n2")
                nc.vector.scalar_tensor_tensor(
                    s2n2[:], u3[:], A2[:], u5[:], OP.mult, OP.add)
                ln2 = s("ln2")
                nc.scalar.activation(ln2[:], s2n2[:], AF.Ln, bias=1e-12)
                n2r = s("n2r")
                nc.scalar.activation(n2r[:], ln2[:], AF.Exp, scale=0.5)
                rd2 = s("rd2")
                nc.vector.reciprocal(rd2[:], den2[:])
                Bz2 = s("Bz2")
                nc.vector.tensor_scalar(Bz2[:], B2[:], Bbc, rd2[:],
                                        OP.mult, OP.mult)
                nc.vector.tensor_scalar(Bz2[:], Bz2[:], BMIN, None, OP.max)
                lnB = s("lnB")
                nc.scalar.activation(lnB[:], Bz2[:], AF.Ln)
                n2 = s("n2")
                nc.vector.tensor_scalar(n2[:], n2r[:], rd2[:], MAXN,
                                        OP.mult, OP.min)
                for db in range(3):
                    nc.tensor.matmul(pT[0:1, 0:256], n2[0:128, 0:1],
                                     brep_sb[:, 0:256],
                                     start=True, stop=True,
                                     tile_position=(0, 0),
                                     skip_group_check=True)
                lnA = s("lnA")
                nc.scalar.activation(lnA[:], n2[:], AF.Ln, bias=1.0)
                rn2r = s("rn2r")
                nc.scalar.activation(rn2r[:], ln2[:], AF.Exp, scale=-0.5)
                a2 = s("a2")
                nc.vector.scalar_tensor_tensor(
                    a2[:], lnA[:], 2.0, lnB[:], OP.mult, OP.subtract)
                L = s("L")
                nc.vector.tensor_scalar(L[:], a2[:], rn2r[:], 0.5,
                                        OP.mult, OP.mult)
                cut = s("cut")
                nc.vector.tensor_scalar(cut[:], L[:], g1[:], aa[:],
                                        OP.mult, OP.mult)

                # --- wide state update: W2 = uG + (cycp/cu)y + (cb/cu)b
                # was built mid-chain; apply cu via the tanh scale slot ----
                tv = rb.tile([128, 256], f32, tag="tv")
                nc.scalar.activation(tv[:], W2[:], AF.Tanh, scale=cut[:])
                s2tp = rb.tile([128, 1], f32, tag="s2tp")
                nc.scalar.activation(scr[:], tv[:], AF.Square,
                                     accum_out=s2tp[:])
                nc.tensor.matmul(psmX[:, 2:3], i4_sb[:], s2tp[:],
                                 start=True, stop=True)
                rnt = s("rnt")
                nc.scalar.activation(rnt[:], psmX[:, 2:3], AF.Ln,
                                     bias=1e-12)
                nc.scalar.activation(rnt[:], rnt[:], AF.Exp, scale=-0.5)
                nc.vector.tensor_scalar(kk2[:], rnt[:], 2.0 * _A999, 2.0,
                                        OP.mult, OP.min)

                # masked last-state accumulate (folded)
                nc.vector.scalar_tensor_tensor(
                    tsel[:], tv[:], eqm_sb[:, t:t + 1], tsel[:],
                    OP.mult, OP.add)
                nc.vector.scalar_tensor_tensor(
                    s2tsel[:], psmX[0:BL, 2:3], eqm_sb[0:BL, t:t + 1],
                    s2tsel[:], OP.mult, OP.add)

                # --- transpose new state (2 full 128x128 blocks) ----------
                nc.tensor.transpose(pT[:, 0:128], tv[:, 0:128], ident_sb[:])
                nc.tensor.transpose(pT[:, 128:256], tv[:, 128:256],
                                    ident_sb[:])
                nc.vector.tensor_copy(tTh[:], pT[:])
                nc.vector.tensor_tensor(tTl[:], pT[:], tTh[:], OP.subtract)

                # inject one deferred P1 tile every 3rd step, at the END
                # of the body so its DVE/PE work queues into the next
                # chain window instead of ahead of this step's reduce ops
                # (tile j lands at step 3(j-8)+1, consumed at step 4j)
                if next_tile < NTILES and t % 3 == 1:
                    p1_tile(next_tile)
                    next_tile += 1
                    if next_tile % 8 == 0 or next_tile == NTILES:
                        p1_chain(next_chain, next_tile)
                        next_chain = next_tile

        # =============== P3: epilogue =======================================
        with (
            tc.tile_pool(name="ebig", bufs=1) as eb,
            tc.tile_pool(name="eps", bufs=1, space="PSUM") as eps,
            tc.tile_pool(name="esc", bufs=2) as es,
        ):
            def e(tag):
                return es.tile([BL, 1], f32, tag=tag, name=tag)

            # last state scalars: ntl = |t_last|, tanh, B_h
            ntl = e("ntl")
            nc.scalar.activation(ntl[:], s2tsel[:], AF.Ln, bias=1e-12)
            nc.scalar.activation(ntl[:], ntl[:], AF.Exp, scale=0.5)
            rntl = e("rntl")
            nc.vector.reciprocal(rntl[:], ntl[:])
            evl = e("evl")
            nc.scalar.activation(evl[:], ntl[:], AF.Exp, scale=2.0)
            nc.vector.tensor_scalar(evl[:], evl[:], 1.0, None, OP.add)
            rq = e("rq")
            nc.vector.reciprocal(rq[:], evl[:])
            tnl = e("tnl")
            nc.vector.tensor_scalar(tnl[:], rq[:], -2.0, 1.0, OP.mult, OP.add)
            nc.vector.tensor_scalar(tnl[:], tnl[:], MAXN, None, OP.min)
            dnl = e("dnl")
            nc.vector.scalar_tensor_tensor(
                dnl[:], rq[:], rq[:], rq[:], OP.mult, OP.subtract)
            Bh = e("Bh")
            nc.vector.tensor_scalar(Bh[:], dnl[:], -4.0, BMIN, OP.mult, OP.max)
            mul_ = e("mul_")
            nc.vector.tensor_tensor(mul_[:], tnl[:], rntl[:], OP.mult)
            kk2l = e("kk2l")
            nc.vector.tensor_scalar(kk2l[:], rntl[:], 2.0 * _A999, 2.0,
                                    OP.mult, OP.min)

            # transpose folded t_last, hyperbolic linear layer
            pT = eps.tile([128, 256], f32, tag="epT")
            nc.tensor.transpose(pT[:, 0:128], tsel[:, 0:128], ident_sb[:])
            nc.tensor.transpose(pT[:, 128:256], tsel[:, 128:256], ident_sb[:])
            tselT = eb.tile([128, 256], f32r, tag="tselT")
            nc.vector.tensor_copy(tselT[:], pT[:])
            wl_sb = []
            for ki in range(8):
                t_ = eb.tile([128, HP1], f32r, tag=f"wl{ki}", name=f"wl{ki}")
                nc.sync.dma_start(t_[:], Wl[ki * 128:(ki + 1) * 128, :])
                wl_sb.append(t_)
            pl = eps.tile([BL, HP1], f32, tag="pl")
            for ki in range(8):
                c0 = (ki % 2) * 128 + (ki // 2) * BL
                for (n0, n1) in ((0, 512), (512, 1024), (1024, 1026)):
                    nc.tensor.matmul(
                        pl[:, n0:n1],
                        tselT[:, c0:c0 + BL],
                        wl_sb[ki][:, n0:n1],
                        start=(ki == 0), stop=(ki == 7))
            scrL = eb.tile([BL, H], f32, tag="scrL")
            s2u = e("es2u")
            nc.scalar.activation(scrL[:], pl[:, 0:H], AF.Square,
                                 accum_out=s2u[:])
            sub = e("esub")
            nc.vector.tensor_copy(sub[:], pl[:, H:H + 1])
            # X = mmatvec(h_last, Wl) folded scalars
            nG = e("enG")
            nc.scalar.activation(nG[:], s2u[:], AF.Ln, bias=1e-12)
            nc.scalar.activation(nG[:], nG[:], AF.Exp, scale=0.5)
            evx = e("evx")
            nc.scalar.activation(evx[:], nG[:], AF.Exp, scale=kk2l[:])
            nc.vector.tensor_scalar(evx[:], evx[:], 1.0, None, OP.add)
            rvx = e("rvx")
            nc.vector.reciprocal(rvx[:], evx[:])
            th = e("eth")
            nc.vector.tensor_scalar(th[:], rvx[:], -2.0, 1.0, OP.mult, OP.add)
            nc.vector.tensor_scalar(th[:], th[:], MAXN, None, OP.min)
            dnx = e("ednx")
            nc.vector.scalar_tensor_tensor(
                dnx[:], rvx[:], rvx[:], rvx[:], OP.mult, OP.subtract)
            B1 = e("eB1")
            nc.vector.tensor_scalar(B1[:], dnx[:], -4.0, BMIN, OP.mult, OP.max)
            x2 = e("ex2")
            nc.vector.tensor_tensor(x2[:], th[:], th[:], OP.mult)
            cwm = e("ecwm")
            nc.vector.reciprocal(cwm[:], nG[:])
            nc.vector.tensor_tensor(cwm[:], cwm[:], th[:], OP.mult)
            # madd(X, bc): y-side is the constant bias point bc
            bc2 = cst_sb[0:BL, C_BC2:C_BC2 + 1]
            Bbc2 = cst_sb[0:BL, C_BBC:C_BBC + 1]
            subt = e("esubt")
            nc.vector.tensor_tensor(subt[:], sub[:], cwm[:], OP.mult)
            A1 = e("eA1")
            nc.vector.tensor_scalar(A1[:], subt[:], 2.0, 1.0, OP.mult, OP.add)
            nc.vector.tensor_scalar(A1[:], A1[:], bc2, None, OP.add)
            ones32e = e("ones32e")
            nc.vector.memset(ones32e[:], 1.0)
            den = e("eden")
            nc.vector.scalar_tensor_tensor(
                den[:], x2[:], bc2, ones32e[:], OP.mult, OP.add)
            nc.vector.scalar_tensor_tensor(
                den[:], subt[:], 2.0, den[:], OP.mult, OP.add)
            aa = e("eaa")
            nc.vector.tensor_tensor(aa[:], A1[:], cwm[:], OP.mult)
            p1 = e("ep1")
            nc.vector.tensor_tensor(p1[:], aa[:], aa[:], OP.mult)
            nc.vector.tensor_tensor(p1[:], p1[:], s2u[:], OP.mult)
            p2 = e("ep2")
            nc.vector.tensor_tensor(p2[:], aa[:], B1[:], OP.mult)
            nc.vector.tensor_tensor(p2[:], p2[:], sub[:], OP.mult)
            nc.vector.tensor_scalar(p2[:], p2[:], 2.0, None, OP.mult)
            p3 = e("ep3")
            nc.vector.tensor_scalar(p3[:], B1[:], B1[:], bc2, OP.mult, OP.mult)
            s2n = e("es2n")
            nc.vector.tensor_tensor(s2n[:], p1[:], p2[:], OP.add)
            nc.vector.tensor_tensor(s2n[:], s2n[:], p3[:], OP.add)
            n1 = e("en1")
            nc.scalar.activation(n1[:], s2n[:], AF.Ln, bias=1e-12)
            nc.scalar.activation(n1[:], n1[:], AF.Exp, scale=0.5)
            rd = e("erd")
            nc.vector.reciprocal(rd[:], den[:])
            gam = e("egam")
            nc.vector.reciprocal(gam[:], n1[:])
            nc.vector.tensor_scalar(gam[:], gam[:], MAXN, None, OP.mult)
            nc.vector.tensor_tensor(gam[:], gam[:], rd[:], OP.min)
            Brepr = e("eBrepr")
            nc.vector.tensor_scalar(Brepr[:], B1[:], Bbc2, rd[:],
                                    OP.mult, OP.mult)
            nc.vector.tensor_scalar(Brepr[:], Brepr[:], BMIN, None, OP.max)
            x2r = e("ex2r")
            nc.vector.tensor_scalar(x2r[:], Brepr[:], -1.0, 1.0,
                                    OP.mult, OP.add)
            c1 = e("ec1")
            nc.vector.tensor_tensor(c1[:], gam[:], A1[:], OP.mult)
            nc.vector.tensor_tensor(c1[:], c1[:], cwm[:], OP.mult)
            c2 = e("ec2")
            nc.vector.tensor_tensor(c2[:], gam[:], B1[:], OP.mult)
            rep = eb.tile([BL, H], f32, tag="rep")
            nc.scalar.activation(rep[:], pl[:, 0:H], AF.Copy, scale=c1[:])
            rep2 = eb.tile([BL, H], f32, tag="rep2")
            nc.vector.scalar_tensor_tensor(
                rep2[:], bcrep_sb[0:BL, :], c2[:], rep[:], OP.mult, OP.add)

            pk = eb.tile([BL, HP2], f32, tag="pk")
            nc.vector.tensor_copy(pk[:, 0:H], rep2[:])
            nc.vector.tensor_copy(pk[:, H:H + 1], x2r[:])
            nc.vector.tensor_copy(pk[:, H + 1:H + 2], Brepr[:])
            nc.sync.dma_start(ccin[:, :], pk[:])
            nc.gpsimd.collective_compute(
                "AllGather", mybir.AluOpType.bypass,
                replica_groups=[list(range(8))],
                ins=[ccin[:, :]], outs=[ccout[:, :]])
            # static full-batch slices: premise rows 0:128, hypothesis 128:256
            ccp = eb.tile([128, HP2], f32, tag="ccp")
            nc.sync.dma_start(ccp[:], ccout[0:128, :])
            cch = eb.tile([128, HP2], f32, tag="cch")
            nc.sync.dma_start(cch[:], ccout[128:256, :])
            prep = ccp[0:128, 0:H]
            hrep = cch[0:128, 0:H]
            px2 = ccp[0:128, H:H + 1]
            hy2c = cch[0:128, H:H + 1]
            pB = ccp[0:128, H + 1:H + 2]
            hB = cch[0:128, H + 1:H + 2]
            ones128 = eb.tile([128, 1], f32, tag="ones128")
            nc.vector.memset(ones128[:], 1.0)

            def f(tag):
                return es.tile([128, 1], f32, tag=tag, name=tag)

            # combine: rep = madd(p_rep, h_rep)
            xyf = f("xyf")
            scrF = eb.tile([128, H], f32, tag="scrF")
            nc.vector.scalar_tensor_tensor(
                scrF[:], prep, 1.0, hrep, OP.mult, OP.mult, accum_out=xyf[:])
            Af = f("Af")
            nc.vector.tensor_scalar(Af[:], xyf[:], 2.0, 1.0, OP.mult, OP.add)
            nc.vector.tensor_scalar(Af[:], Af[:], hy2c, None, OP.add)
            Bf = f("Bf")
            nc.vector.tensor_scalar(Bf[:], px2, -1.0, 1.0, OP.mult, OP.add)
            denf = f("denf")
            nc.vector.scalar_tensor_tensor(
                denf[:], px2, hy2c, ones128[:], OP.mult, OP.add)
            nc.vector.scalar_tensor_tensor(
                denf[:], xyf[:], 2.0, denf[:], OP.mult, OP.add)
            numf = eb.tile([128, H], f32, tag="numf")
            nc.scalar.activation(numf[:], prep, AF.Copy, scale=Af[:])
            numf2 = eb.tile([128, H], f32, tag="numf2")
            nc.vector.scalar_tensor_tensor(
                numf2[:], hrep, Bf[:], numf[:], OP.mult, OP.add)
            s2f = f("s2f")
            nc.scalar.activation(scrF[:], numf2[:], AF.Square, accum_out=s2f[:])
            nf = f("nf")
            nc.scalar.activation(nf[:], s2f[:], AF.Ln, bias=1e-12)
            nc.scalar.activation(nf[:], nf[:], AF.Exp, scale=0.5)
            rdf = f("rdf")
            nc.vector.reciprocal(rdf[:], denf[:])
            gf = f("gf")
            nc.vector.reciprocal(gf[:], nf[:])
            nc.vector.tensor_scalar(gf[:], gf[:], MAXN, None, OP.mult)
            nc.vector.tensor_tensor(gf[:], gf[:], rdf[:], OP.min)
            Brf = f("Brf")
            nc.vector.tensor_scalar(Brf[:], pB, hB, rdf[:], OP.mult, OP.mult)
            nc.vector.tensor_scalar(Brf[:], Brf[:], BMIN, None, OP.max)
            y2f = f("y2f")
            nc.vector.tensor_scalar(y2f[:], Brf[:], -1.0, 1.0, OP.mult, OP.add)
            repf = eb.tile([128, H], f32, tag="repf")
            nc.scalar.activation(repf[:], numf2[:], AF.Copy, scale=gf[:])

            # MLR
            pT2 = eps.tile([128, H], f32, tag="epT2")
            for c in range(8):
                nc.tensor.transpose(
                    pT2[:, c * 128:(c + 1) * 128],
                    repf[:, c * 128:(c + 1) * 128],
                    ident_sb[:])
            repT = eb.tile([128, H], f32r, tag="repT")
            nc.vector.tensor_copy(repT[:], pT2[:])
            mlr_sb = []
            for ki in range(8):
                t_ = eb.tile([128, 2 * NCLS], f32r, tag=f"mlr{ki}",
                             name=f"mlr{ki}")
                nc.sync.dma_start(t_[:], mlrT[ki * 128:(ki + 1) * 128, :])
                mlr_sb.append(t_)
            pm = eps.tile([128, 2 * NCLS], f32, tag="pm")
            for ki in range(8):
                nc.tensor.matmul(
                    pm[:, :],
                    repT[:, ki * 128:(ki + 1) * 128],
                    mlr_sb[ki][:, :],
                    start=(ki == 0), stop=(ki == 7))

            def e3(tag):
                return es.tile([128, NCLS], f32, tag=tag, name=tag)

            xp = e3("xp")
            nc.vector.tensor_copy(xp[:], pm[:, 0:NCLS])
            xa = e3("xa")
            nc.vector.tensor_copy(xa[:], pm[:, NCLS:2 * NCLS])
            pk2 = cst_sb[:, C_PK2:C_PK2 + NCLS]
            pkak = cst_sb[:, C_PKAK:C_PKAK + NCLS]
            anc = cst_sb[:, C_AN:C_AN + NCLS]
            lamc = cst_sb[:, C_LAM:C_LAM + NCLS]
            Bpk = cst_sb[:, C_BPK:C_BPK + NCLS]
            xyk = e3("xyk")
            nc.vector.tensor_scalar(xyk[:], xp[:], -1.0, None, OP.mult)
            Ak = e3("Ak")
            nc.vector.tensor_scalar(Ak[:], xyk[:], 2.0, 1.0, OP.mult, OP.add)
            nc.vector.tensor_scalar(Ak[:], Ak[:], y2f[:], None, OP.add)
            Bk = e3("Bk")
            nc.vector.tensor_scalar(Bk[:], pk2, -1.0, 1.0, OP.mult, OP.add)
            denk = e3("denk")
            nc.vector.tensor_scalar(denk[:], pk2, y2f[:], None, OP.mult)
            nc.vector.tensor_tensor(denk[:], denk[:], xyk[:], OP.add)
            nc.vector.tensor_tensor(denk[:], denk[:], xyk[:], OP.add)
            nc.vector.tensor_scalar(denk[:], denk[:], 1.0, None, OP.add)
            q1k = e3("q1k")
            nc.vector.tensor_tensor(q1k[:], Ak[:], Ak[:], OP.mult)
            nc.vector.tensor_tensor(q1k[:], q1k[:], pk2, OP.mult)
            q2k = e3("q2k")
            nc.vector.tensor_tensor(q2k[:], Ak[:], Bk[:], OP.mult)
            nc.vector.tensor_tensor(q2k[:], q2k[:], xyk[:], OP.mult)
            nc.vector.tensor_scalar(q2k[:], q2k[:], 2.0, None, OP.mult)
            q3k = e3("q3k")
            nc.vector.tensor_tensor(q3k[:], Bk[:], Bk[:], OP.mult)
            nc.vector.tensor_scalar(q3k[:], q3k[:], y2f[:], None, OP.mult)
            s2k = e3("s2k")
            nc.vector.tensor_tensor(s2k[:], q1k[:], q2k[:], OP.add)
            nc.vector.tensor_tensor(s2k[:], s2k[:], q3k[:], OP.add)
            n1k = e3("n1k")
            nc.scalar.activation(n1k[:], s2k[:], AF.Ln, bias=1e-12)
            nc.scalar.activation(n1k[:], n1k[:], AF.Exp, scale=0.5)
            rdk = e3("rdk")
            nc.vector.reciprocal(rdk[:], denk[:])
            gk = e3("gk")
            nc.vector.reciprocal(gk[:], n1k[:])
            nc.vector.tensor_scalar(gk[:], gk[:], MAXN, None, OP.mult)
            nc.vector.tensor_tensor(gk[:], gk[:], rdk[:], OP.min)
            # 1 - |z|^2 via identity: Bz = max(Bpk * Brf * rdk, BMIN)
            Bzk = e3("Bzk")
            nc.vector.tensor_scalar(Bzk[:], Bpk, Brf[:], None, OP.mult)
            nc.vector.tensor_tensor(Bzk[:], Bzk[:], rdk[:], OP.mult)
            nc.vector.tensor_scalar(Bzk[:], Bzk[:], BMIN, None, OP.max)
            zak = e3("zak")
            nc.vector.tensor_tensor(zak[:], Ak[:], pkak, OP.mult)
            nc.vector.tensor_scalar(zak[:], zak[:], -1.0, None, OP.mult)
            q4k = e3("q4k")
            nc.vector.tensor_tensor(q4k[:], Bk[:], xa[:], OP.mult)
            nc.vector.tensor_tensor(zak[:], zak[:], q4k[:], OP.add)
            nc.vector.tensor_tensor(zak[:], zak[:], gk[:], OP.mult)
            vk = e3("vk")
            nc.vector.tensor_tensor(vk[:], Bzk[:], anc, OP.mult)
            nc.vector.reciprocal(vk[:], vk[:])
            nc.vector.tensor_tensor(vk[:], vk[:], zak[:], OP.mult)
            nc.vector.tensor_scalar(vk[:], vk[:], 2.0, None, OP.mult)
            # asinh(v) = sign(v) * ln(|v| + sqrt(v^2+1))  (cancellation-free)
            av = e3("av")
            nc.scalar.activation(av[:], vk[:], AF.Abs)
            sg = e3("sg")
            nc.scalar.activation(sg[:], vk[:], AF.Sign)
            sq = e3("sq")
            nc.vector.tensor_tensor(sq[:], vk[:], vk[:], OP.mult)
            nc.vector.tensor_scalar(sq[:], sq[:], 1.0, None, OP.add)
            nc.scalar.activation(sq[:], sq[:], AF.Ln)
            nc.scalar.activation(sq[:], sq[:], AF.Exp, scale=0.5)
            nc.vector.tensor_tensor(sq[:], sq[:], av[:], OP.add)
            nc.scalar.activation(sq[:], sq[:], AF.Ln)
            nc.vector.tensor_tensor(sq[:], sq[:], sg[:], OP.mult)
            logit = e3("logit")
            nc.vector.tensor_tensor(logit[:], sq[:], anc, OP.mult)
            nc.vector.tensor_tensor(logit[:], logit[:], lamc, OP.mult)
            nc.sync.dma_start(out[:, :], logit[:])

    nc.finalize()
    return nc


def _bf16_pair(x):
    """Split fp32 array into (hi, lo) bf16 pieces: hi = bf16(x),
    lo = bf16(x - hi)."""
    from concourse import mybir
    bd = mybir.dt.np(mybir.dt.bfloat16)
    hi = x.astype(np.float32).astype(bd)
    lo = (x.astype(np.float32) - hi.astype(np.float32)).astype(bd)
    return hi, lo


def _host_prep(inputs, s_steps=S):
    f = np.float32
    emb = inputs["emb_table"].astype(f)
    in_maps = []
    for c in range(8):
        q, r = c % 4, c // 4
        ids = (inputs["premise"] if r == 0 else inputs["hypothesis"])[
            q * BL:(q + 1) * BL, :s_steps].astype(np.int64)
        lens = (inputs["p_len"] if r == 0 else inputs["h_len"])[
            q * BL:(q + 1) * BL].astype(np.int64)
        lens = np.minimum(lens, s_steps)
        U = (inputs["Up"] if r == 0 else inputs["Uh"]).astype(f)
        W = (inputs["Wp"] if r == 0 else inputs["Wh"]).astype(f)
        bv = (inputs["bp"] if r == 0 else inputs["bh"]).astype(f)
        Wlin = (inputs["Wcp"] if r == 0 else inputs["Wch"]).astype(f)
        bc = (inputs["bcp"] if r == 0 else inputs["bch"]).astype(f)
        a_mlr = inputs["a_mlr"].astype(f)
        p_mlr = inputs["p_mlr"].astype(f)

        x = emb[ids]
        xTv = np.ascontiguousarray(x.transpose(2, 1, 0).reshape(E, s_steps * BL))
        xTh_v, xTl_v = _bf16_pair(xTv)
        xn2 = np.maximum((x.astype(np.float64) ** 2).sum(-1), 1e-12)
        xn2v = np.ascontiguousarray(
            xn2.transpose(1, 0).reshape(-1, 128).T).astype(f)
        zc = np.zeros((H, 1), f)
        zc2 = np.zeros((H, 2), f)
        Ucv = np.concatenate([U, (U @ bv)[:, None], np.zeros((E, 1), f)], 1)
        # W packed per col-group g: [W[:,256g:256g+256] | bcol | pad] where
        # bcol = W@b only in group 3, zeros elsewhere.
        Wb = (W @ bv)[:, None]
        Wrv = np.concatenate(
            [W[:, 0:256], zc2, W[:, 256:512], zc2, W[:, 512:768], zc2,
             W[:, 768:1024], Wb, zc], 1)
        Wlv = np.concatenate([Wlin, (Wlin @ bc)[:, None], zc], 1)
        Uch_v, Ucl_v = _bf16_pair(Ucv)
        Wrh_v, Wrl_v = _bf16_pair(Wrv)
        b2 = float(bv.astype(np.float64) @ bv.astype(np.float64))
        bc2 = float(bc.astype(np.float64) @ bc.astype(np.float64))
        p2 = np.sum(p_mlr.astype(np.float64) ** 2, -1)
        cstv = np.zeros((128, CSTW), f)
        cstv[:, C_OPB2] = 1.0 + b2
        cstv[:, C_NB2] = -b2
        cstv[:, C_B2] = b2
        cstv[:, C_BB] = 1.0 - b2
        cstv[:, C_BC2] = bc2
        cstv[:, C_BBC] = 1.0 - bc2
        cstv[:, C_PK2:C_PK2 + NCLS] = p2
        cstv[:, C_PKAK:C_PKAK + NCLS] = np.sum(
            p_mlr.astype(np.float64) * a_mlr.astype(np.float64), -1)
        cstv[:, C_AN:C_AN + NCLS] = np.sqrt(
            np.maximum(np.sum(a_mlr.astype(np.float64) ** 2, -1), 1e-12))
        cstv[:, C_LAM:C_LAM + NCLS] = 2.0 / np.maximum(1.0 - p2, 1e-5)
        cstv[:, C_BPK:C_BPK + NCLS] = 1.0 - p2
        eqv = (np.arange(s_steps)[None, :] == (lens - 1)[:, None]).astype(f)
        brep_f = np.tile(bv.reshape(4, 256)[:, None, :],
                         (1, BL, 1)).reshape(128, 256).astype(f)
        in_maps.append({
            "xTh": xTh_v, "xTl": xTl_v, "xn2d": xn2v,
            "Uch": Uch_v, "Ucl": Ucl_v, "Wrh": Wrh_v, "Wrl": Wrl_v,
            "Wl": Wlv,
            "brep": brep_f,
            "bcrep": np.broadcast_to(bc, (128, H)).copy(),
            "mlrT": np.concatenate([p_mlr.T, a_mlr.T], 1).astype(f),
            "cst": cstv, "eqm": np.tile(eqv, (4, 1)).astype(f),
            "ident": np.eye(128, dtype=f),
            "i4m": np.tile(np.eye(BL, dtype=f), (4, 4)),
        })
    return in_maps


def kernel(premise, p_len, hypothesis, h_len, emb_table, Wp, Up, bp,
           Wh, Uh, bh, Wcp, bcp, Wch, bch, a_mlr, p_mlr,
           s_steps=S, trace=False):
    from concourse.bass_utils import run_bass_kernel_spmd
    inputs = dict(premise=premise, p_len=p_len, hypothesis=hypothesis,
                  h_len=h_len, emb_table=emb_table, Wp=Wp, Up=Up, bp=bp,
                  Wh=Wh, Uh=Uh, bh=bh, Wcp=Wcp, bcp=bcp, Wch=Wch, bch=bch,
                  a_mlr=a_mlr, p_mlr=p_mlr)
    inputs = {k: np.asarray(v) for k, v in inputs.items()}
    if s_steps not in _CACHE:
        _CACHE[s_steps] = _build(s_steps)
    nc = _CACHE[s_steps]
    in_maps = _host_prep(inputs, s_steps)
    res = run_bass_kernel_spmd(nc, in_maps, core_ids=list(range(8)),
                               trace=trace)
    kernel.last_results = res
    return res.results[0]["out"].astype(np.float32)


kernel.last_results = None



# revision 44
# speedup vs baseline: 1.1885x; 1.0075x over previous
"""Trainium2 Bass kernel for the hyperbolic (Poincare-ball) AddRNN NLI model.

Sharding: 8 cores SPMD. Core c: RNN role r=c//4 (0=premise, 1=hypothesis),
batch quarter q=c%4 (rows 32q..32q+31). Each core runs the full sequential
scan for its (role, quarter).

Architecture (beyond the original folded-scalar implementation):
- Folded-H layout: every wide [32,1024] tensor lives as [128,256], with
  partition 32j+b holding batch row b's H-chunk j. All wide DVE/ACT ops
  (square-accums, state update, tanh, masked accumulates) run 4x faster
  than batch-major, and the per-step state transpose is 2 full 128x128 PE
  transposes instead of 8 thin ones.
- The recurrent W-matmul runs as 4 concurrent PE column-groups
  (tile_position=(0,32j)), group j computing H cols [256j,256j+258) of
  t@W straight into its folded PSUM strip; 96 bf16 N=258 matmuls/step at
  ~110ns per 4-way group when warm. bf16 hi/lo 3-pass GEMM (th@Wh + th@Wl
  + tl@Wh) keeps per-dot error ~2^-18 (fp32r single-pass amplifies to
  ~2e-2 over 256 steps - too close to the gate). The W@b bias-dot rides
  as column 256 of group 3's weight tile.
- Cross-strip reductions (|uG|^2, <uG,y>, <uG,Wb>, |t|^2) go through tiny
  fp32 matmuls with I4rep = tile(I32,(4,4)) which both sum the 4 strips
  AND replicate to all 128 partitions, so the whole Mobius scalar chain
  runs on [128,1] tiles and its outputs feed the folded state update
  directly (no broadcast). Three decoupled partial->reduce paths let the
  chain's first Ln wait only on the |uG|^2 path.
- P1 (per-token y = mmatvec(x,U) GEMM) is interleaved into the scan: one
  128-token tile per 3 steps plus chunked table-chains, hiding ~0.45ms of
  prologue inside the scan's serial-latency gaps (Tile tracks the
  ydr DRAM RAW deps). P1 writes ydr pre-folded via a 3-level DMA AP.
- HAM keep-warm: 3 bursts of 3 fp32 dummy matmuls, data-dependent on
  mid-chain scalars, keep the PE activity monitor at K=8/8 through the
  ~9us scalar-chain window; without them the whole matmul phase runs at
  1.2GHz (measured ~5.3us vs ~2.9us warm).
- ACT tables pinned to natural_log_exp_and_others + sigmoid_and_others
  (2 hidden table loads/step). Cancellation-free gyro algebra:
  1-|x(+)y|^2 = (1-|x|^2)(1-|y|^2)/den through both madds;
  1-tanh^2 = 4rv(1-rv); den1 = A1 - y2*B1 (uses 1-th^2 == B1 exactly,
  clamps included); den2 = A2 - b2*B2; fused tensor_scalar/stt forms
  throughout; artanh via Ln(n2, bias=1) - Ln(Bz2).
- Late-chain shortcut: the wide update W2 = uG + (cycp/cu)y + (cb/cu)b is
  built MID-chain (the ratios cycp/cu = B1*cyt/aa and cb/cu =
  B2/(A2*gam*aa) need no late chain values), and cu is applied through
  the tanh's free scale slot - removing cycp->w0->w1->w2 (~1.6us) from
  the serial tail. P1 tiles are injected at the END of the step body so
  their DVE copies queue into the next chain window, not ahead of the
  reduce ops; w0's old ACT Copy is gone so the tanh table-load no longer
  stalls TANH in the ACT FIFO.
Per-step ~16.2us: matmul ~2.8 + reduce ~1.3 + scalar chain ~9 +
tanh/transpose ~2. History: 8.36ms baseline -> 5.88 (folding +
col-tiling) -> 5.25 (replicated chain, P1 interleave, fusions) -> 4.78
(HAM keep-warm bursts) -> 4.55 (cu-scale tail cut) -> 4.54ms (early lnB
so the tanh table-load starts sooner; cut = L*g1*aa drops the m2 op)
-> 4.537ms (P1 injection spread to every 3rd step). Tried+reverted: epilogue weight preload into cpool (+3.7us/step SBUF
shuffle), split CAST per transpose block (+0.75us/step), moving the
tsel/s2tsel accumulates after CAST/SUB (+0.8ms nonlocal scheduler
effect).
"""
import numpy as np

B, S, E, H, V, NCLS = 128, 256, 300, 1024, 32000, 3
BL = 32
MAXN = 1.0 - 1e-3
BMIN = float(1.0 - np.float64(MAXN) * np.float64(MAXN))  # 1-MAXN^2
_A999 = float(np.arctanh(np.float64(MAXN)))

_CACHE = {}

# cst column map
C_B2, C_BB, C_BC2, C_BBC = 0, 1, 2, 3
C_PK2, C_PKAK, C_AN, C_LAM, C_BPK = 4, 7, 10, 13, 16
C_OPB2, C_NB2 = 19, 20
CSTW = 21


def _make_bacc():
    """Bacc with activation-table insertion pinned to two sets so the
    Ln/Exp/Square/Copy cluster and the wide Tanh never thrash tables."""
    from concourse import bacc
    from concourse.hw_specs import get_activation_tables
    import bass_rust as _bass_rust

    class _Bacc2(bacc.Bacc):
        def insert_act_table_loads(self):
            from concourse import mybir as mb
            has_activation = any(
                isinstance(i, mb.InstActivation)
                for b in self.main_func.blocks
                for i in b.instructions
            )
            if not has_activation:
                return
            # Keep the canonical list (act_func_set_id is positional into
            # act_info.json) but empty out every set except the two we pin,
            # so the chooser can only bind activations to those.
            tabs = get_activation_tables(self.m.arch)
            keep = ("natural_log_exp_and_others", "sigmoid_and_others")
            tables = [(k, v if k in keep else set()) for k, v in tabs.items()]
            _bass_rust.insert_act_table_loads(self, tables)

    return _Bacc2()


def _build(s_steps, use_gpsimd_sel=None):
    import os
    if use_gpsimd_sel is None:
        use_gpsimd_sel = os.environ.get("V2_GPSEL", "1") == "1"
    use_f32r = os.environ.get("V2_F32R", "1") == "1"
    use_tabs = os.environ.get("V2_TABS", "1") == "1"

    import concourse.tile as tile
    from concourse import mybir

    f32 = mybir.dt.float32
    f32r = mybir.dt.float32r if use_f32r else mybir.dt.float32
    bf16 = mybir.dt.bfloat16
    AF = mybir.ActivationFunctionType
    OP = mybir.AluOpType

    NT = BL * s_steps
    NTILES = NT // 128
    HP1 = H + 2  # weights padded to even width for fp32r ISA rules
    HP2 = H + 2  # allgather payload: rep | x2 | B

    if use_tabs:
        nc = _make_bacc()
    else:
        from concourse import bacc as _bacc
        nc = _bacc.Bacc()

    # register the Ln-bias constant as a const AP (only 0.0/1.0 are built in)
    _cb = nc.alloc_sbuf_tensor("const-f32-1em12", [128, 1], f32)
    nc.gpsimd.memset(_cb.ap(), 1e-12)
    nc.const_aps.aps[(f32, 1e-12)] = _cb.ap()
    nc.all_engine_barrier()

    GW = 258          # per-col-group W width: 256 H cols + b col + pad
    xTh = nc.declare_dram_parameter("xTh", [E, NT], bf16, isOutput=False)
    xTl = nc.declare_dram_parameter("xTl", [E, NT], bf16, isOutput=False)
    xn2d = nc.declare_dram_parameter("xn2d", [128, NTILES], f32, isOutput=False)
    Uch = nc.declare_dram_parameter("Uch", [E, HP1], bf16, isOutput=False)
    Ucl = nc.declare_dram_parameter("Ucl", [E, HP1], bf16, isOutput=False)
    Wrh = nc.declare_dram_parameter("Wrh", [H, 4 * GW], bf16, isOutput=False)
    Wrl = nc.declare_dram_parameter("Wrl", [H, 4 * GW], bf16, isOutput=False)
    Wl = nc.declare_dram_parameter("Wl", [H, HP1], f32r, isOutput=False)
    brep = nc.declare_dram_parameter("brep", [128, 256], f32, isOutput=False)
    bcrep = nc.declare_dram_parameter("bcrep", [128, H], f32, isOutput=False)
    mlrT = nc.declare_dram_parameter("mlrT", [H, 2 * NCLS], f32r, isOutput=False)
    cst = nc.declare_dram_parameter("cst", [128, CSTW], f32, isOutput=False)
    eqm = nc.declare_dram_parameter("eqm", [128, s_steps], f32, isOutput=False)
    ident = nc.declare_dram_parameter("ident", [128, 128], f32, isOutput=False)
    i4m = nc.declare_dram_parameter("i4m", [128, 128], f32, isOutput=False)
    out = nc.declare_dram_parameter("out", [B, NCLS], f32, isOutput=True)

    ydr = nc.dram_tensor("ydr", [s_steps * 128, 256], f32)
    ccin = nc.dram_tensor("ccin", [BL, HP2], f32)
    ccout = nc.dram_tensor("ccout", [8 * BL, HP2], f32, addr_space="Shared")

    with tile.TileContext(nc) as tc:
      with tc.tile_pool(name="const", bufs=1) as cpool:
        # ---------------- constants to SBUF --------------------------------
        wh_sb, wl_sb = [], []
        for ki in range(8):
            th_ = cpool.tile([128, 4 * GW], bf16, tag=f"wh{ki}", name=f"wh{ki}")
            nc.sync.dma_start(th_[:], Wrh[ki * 128:(ki + 1) * 128, :])
            wh_sb.append(th_)
            tl_ = cpool.tile([128, 4 * GW], bf16, tag=f"wl{ki}b", name=f"wl{ki}b")
            nc.sync.dma_start(tl_[:], Wrl[ki * 128:(ki + 1) * 128, :])
            wl_sb.append(tl_)
        uh_sb, ul_sb = [], []
        for c, kc in enumerate((128, 128, E - 256)):
            th_ = cpool.tile([128, HP1], bf16, tag=f"uh{c}", name=f"uh{c}")
            nc.sync.dma_start(th_[0:kc, :], Uch[c * 128:c * 128 + kc, :])
            uh_sb.append(th_)
            tl_ = cpool.tile([128, HP1], bf16, tag=f"ul{c}", name=f"ul{c}")
            nc.sync.dma_start(tl_[0:kc, :], Ucl[c * 128:c * 128 + kc, :])
            ul_sb.append(tl_)
        brep_sb = cpool.tile([128, 256], f32, tag="brep")
        nc.sync.dma_start(brep_sb[:], brep[:, :])
        bcrep_sb = cpool.tile([128, H], f32, tag="bcrep")
        nc.sync.dma_start(bcrep_sb[:], bcrep[:, :])
        cst_sb = cpool.tile([128, CSTW], f32, tag="cst")
        nc.sync.dma_start(cst_sb[:], cst[:, :])
        ident_sb = cpool.tile([128, 128], f32, tag="ident")
        nc.sync.dma_start(ident_sb[:], ident[:, :])
        i4_sb = cpool.tile([128, 128], f32, tag="i4m")
        nc.sync.dma_start(i4_sb[:], i4m[:, :])
        eqm_sb = cpool.tile([128, s_steps], f32, tag="eqm")
        nc.sync.dma_start(eqm_sb[:], eqm[:, :])
        xn2a = cpool.tile([128, NTILES], f32, tag="xn2a")
        nc.sync.dma_start(xn2a[:], xn2d[:, :])
        un2a = cpool.tile([128, NTILES], f32, tag="un2a")
        uba = cpool.tile([128, NTILES], f32, tag="uba")
        # per-step tables, replicated x4 along partitions: [128, s_steps]
        y2s = cpool.tile([128, s_steps], f32, tag="y2s")
        y2p1s = cpool.tile([128, s_steps], f32, tag="y2p1s")
        Bys = cpool.tile([128, s_steps], f32, tag="Bys")
        cys = cpool.tile([128, s_steps], f32, tag="cys")
        sybs = cpool.tile([128, s_steps], f32, tag="sybs")
        ny2s = cpool.tile([128, s_steps], f32, tag="ny2s")

        b2c = cst_sb[:, C_B2:C_B2 + 1]
        Bbc = cst_sb[:, C_BB:C_BB + 1]
        opb2c = cst_sb[:, C_OPB2:C_OPB2 + 1]
        nb2c = cst_sb[:, C_NB2:C_NB2 + 1]

        # =============== P1: prologue  y = mmatvec(x, U) ====================
        # P1 (prologue GEMM y = mmatvec(x,U)) and P2 (the scan) share one
        # pool scope: P1 tiles 8..NTILES-1 and the chunked table-chains are
        # emitted INSIDE the step loop (one tile per 2 steps), so their
        # PE/DVE/ACT work hides in the scan's serial-latency gaps. DRAM RAW
        # deps (ydr write -> yt read) are tracked by Tile, order is safe.
        with (
            tc.tile_pool(name="bigio", bufs=2) as bpool,
            tc.tile_pool(name="psA", bufs=1, space="PSUM") as psA,
            tc.tile_pool(name="scal", bufs=2) as spool,
            tc.tile_pool(name="state", bufs=1) as stp,
            tc.tile_pool(name="rbig", bufs=2) as rb,
            tc.tile_pool(name="rps", bufs=2, space="PSUM") as rps,
            tc.tile_pool(name="rpsT", bufs=1, space="PSUM") as rpsT,
            tc.tile_pool(name="rpsS", bufs=1, space="PSUM") as rpsS,
            tc.tile_pool(name="rsc", bufs=3) as rs,
        ):
            CH = ((0, 512), (512, 1024), (1024, 1026))

            def p1_tile(j):
                xkh = bpool.tile([128, 3 * 128], bf16, tag="xkh")
                nc.sync.dma_start(
                    xkh[:, 0:256].rearrange("p (c m) -> p c m", c=2),
                    xTh[0:256, j * 128:(j + 1) * 128].rearrange(
                        "(c p) m -> p c m", c=2))
                nc.sync.dma_start(
                    xkh[0:E - 256, 256:384],
                    xTh[256:E, j * 128:(j + 1) * 128])
                xkl = bpool.tile([128, 3 * 128], bf16, tag="xkl")
                nc.sync.dma_start(
                    xkl[:, 0:256].rearrange("p (c m) -> p c m", c=2),
                    xTl[0:256, j * 128:(j + 1) * 128].rearrange(
                        "(c p) m -> p c m", c=2))
                nc.sync.dma_start(
                    xkl[0:E - 256, 256:384],
                    xTl[256:E, j * 128:(j + 1) * 128])

                pu = psA.tile([128, HP1], f32, tag="pu")
                for c, kc in enumerate((128, 128, E - 256)):
                    for (n0, n1) in CH:
                        nc.tensor.matmul(
                            pu[:, n0:n1],
                            xkh[0:kc, c * 128:c * 128 + 128],
                            uh_sb[c][0:kc, n0:n1],
                            start=(c == 0), stop=False)
                    for (n0, n1) in CH:
                        nc.tensor.matmul(
                            pu[:, n0:n1],
                            xkh[0:kc, c * 128:c * 128 + 128],
                            ul_sb[c][0:kc, n0:n1],
                            start=False, stop=False)
                for c, kc in enumerate((128, 128, E - 256)):
                    for (n0, n1) in CH:
                        nc.tensor.matmul(
                            pu[:, n0:n1],
                            xkl[0:kc, c * 128:c * 128 + 128],
                            uh_sb[c][0:kc, n0:n1],
                            start=False, stop=(c == 2))
                ysc = bpool.tile([128, H], f32, tag="ysc")
                nc.vector.tensor_copy(ysc[:], pu[:, 0:H])
                nc.vector.tensor_copy(uba[:, j:j + 1], pu[:, H:H + 1])
                # folded scatter: ydr[(4j+sl)*128 + jj*32 + b, c] =
                #   y[b, 4j+sl][256*jj + c]
                for sl in range(4):
                    nc.sync.dma_start(
                        ydr[(j * 4 + sl) * 128:(j * 4 + sl + 1) * 128,
                            :].rearrange("(jj b) c -> b jj c", jj=4, b=BL),
                        ysc[sl * BL:(sl + 1) * BL, :].rearrange(
                            "b (jj c) -> b jj c", jj=4))
                scrH = bpool.tile([128, H], f32, tag="scrH")
                nc.scalar.activation(scrH[:], ysc[:], AF.Square,
                                     accum_out=un2a[:, j:j + 1])

            def p1_chain(c0, c1):
                # chunked prologue scalar chain on [128, c1-c0]
                W_ = c1 - c0

                def sc(tag):
                    return spool.tile([128, W_], f32, tag=f"{tag}_{W_}",
                                      name=f"{tag}_{c0}")

                xn = sc("p_xn")
                nc.scalar.activation(xn[:], xn2a[:, c0:c1], AF.Ln, bias=1e-12)
                nc.scalar.activation(xn[:], xn[:], AF.Exp, scale=0.5)
                rxn = sc("p_rxn")
                nc.vector.reciprocal(rxn[:], xn[:])
                pxa = sc("p_pxa")
                nc.vector.tensor_scalar(pxa[:], xn[:], 1.0, None, OP.add)
                mxa = sc("p_mxa")
                nc.vector.tensor_scalar(mxa[:], xn[:], -1.0, 1.0,
                                        OP.mult, OP.add)
                nc.vector.reciprocal(mxa[:], mxa[:])
                nc.vector.tensor_tensor(pxa[:], pxa[:], mxa[:], OP.mult)
                lnr = sc("p_lnr")
                nc.scalar.activation(lnr[:], pxa[:], AF.Ln)  # = 2*artanh(xn)
                un = sc("p_un")
                nc.scalar.activation(un[:], un2a[:, c0:c1], AF.Ln, bias=1e-12)
                nc.scalar.activation(un[:], un[:], AF.Exp, scale=0.5)
                run = sc("p_run")
                nc.vector.reciprocal(run[:], un[:])
                arg = sc("p_arg")
                nc.vector.tensor_tensor(arg[:], un[:], rxn[:], OP.mult)
                nc.vector.tensor_tensor(arg[:], arg[:], lnr[:], OP.mult)
                # arg = 2*(un/xn)*artanh(xn); tanh = 1-2/(e^arg+1)
                ev = sc("p_ev")
                nc.scalar.activation(ev[:], arg[:], AF.Exp)
                nc.vector.tensor_scalar(ev[:], ev[:], 1.0, None, OP.add)
                rv = sc("p_rv")
                nc.vector.reciprocal(rv[:], ev[:])
                th = sc("p_th")
                nc.vector.tensor_scalar(th[:], rv[:], -2.0, 1.0,
                                        OP.mult, OP.add)
                nc.vector.tensor_scalar(th[:], th[:], MAXN, None, OP.min)
                y2a = sc("p_y2a")
                nc.vector.tensor_tensor(y2a[:], th[:], th[:], OP.mult)
                y2p1a = sc("p_y2p1a")
                nc.vector.tensor_scalar(y2p1a[:], y2a[:], 1.0, None, OP.add)
                dneg = sc("p_dneg")
                nc.vector.tensor_tensor(dneg[:], rv[:], rv[:], OP.mult)
                nc.vector.tensor_tensor(dneg[:], dneg[:], rv[:], OP.subtract)
                Bya = sc("p_Bya")
                nc.vector.tensor_scalar(Bya[:], dneg[:], -4.0, BMIN,
                                        OP.mult, OP.max)
                cya = sc("p_cya")
                nc.vector.tensor_tensor(cya[:], th[:], run[:], OP.mult)
                syba = sc("p_syba")
                nc.vector.tensor_tensor(syba[:], cya[:], uba[:, c0:c1],
                                        OP.mult)
                ny2a = sc("p_ny2a")
                nc.vector.tensor_scalar(ny2a[:], y2a[:], -1.0, None, OP.mult)
                for g in range(4):
                    for jj in range(4):
                        dst = slice(jj * BL, (jj + 1) * BL)
                        sg = slice(g * BL, (g + 1) * BL)
                        cs = slice(4 * c0 + g, 4 * c1, 4)
                        nc.sync.dma_start(y2s[dst, cs], y2a[sg, :])
                        nc.sync.dma_start(y2p1s[dst, cs], y2p1a[sg, :])
                        nc.sync.dma_start(Bys[dst, cs], Bya[sg, :])
                        nc.sync.dma_start(cys[dst, cs], cya[sg, :])
                        nc.sync.dma_start(sybs[dst, cs], syba[sg, :])
                        nc.sync.dma_start(ny2s[dst, cs], ny2a[sg, :])

            # prefix: first 8 tiles + their tables (feeds steps 0..31)
            PRE = min(8, NTILES)
            for j in range(PRE):
                p1_tile(j)
            p1_chain(0, PRE)

            tTh = stp.tile([128, 256], bf16, tag="tTh")
            nc.vector.memset(tTh[:].bitcast(mybir.dt.uint16), 0)
            tTl = stp.tile([128, 256], bf16, tag="tTl")
            nc.vector.memset(tTl[:].bitcast(mybir.dt.uint16), 0)
            kk2 = stp.tile([128, 1], f32, tag="kk2")
            nc.vector.memset(kk2[:], 2.0)
            tsel = cpool.tile([128, 256], f32, tag="tsel", name="tsel")
            nc.vector.memset(tsel[:], 0.0)
            s2tsel = cpool.tile([BL, 1], f32, tag="s2tsel", name="s2tsel")
            nc.vector.memset(s2tsel[:], 0.0)

            def stat(tt, ki):
                c0 = (ki % 2) * 128 + (ki // 2) * BL
                return tt[:, c0:c0 + BL]

            def s(tag):
                return rs.tile([128, 1], f32, tag=tag, name=tag)

            next_tile = PRE
            next_chain = PRE
            for t in range(s_steps):
                yt = rb.tile([128, 256], f32, tag="yt")
                nc.sync.dma_start(yt[:], ydr[t * 128:(t + 1) * 128, :])
                pT = rpsT.tile([128, 256], f32, tag="pT")

                # --- W matmul: 4 col-groups concurrent, k-outer -----------
                pu = rps.tile([128, GW], f32, tag="rpu")
                for ki in range(8):
                    sh = stat(tTh, ki)
                    for g in range(4):
                        nc.tensor.matmul(
                            pu[g * BL:(g + 1) * BL, :], sh,
                            wh_sb[ki][:, g * GW:(g + 1) * GW],
                            start=(ki == 0), stop=False,
                            tile_position=(0, g * BL))
                        nc.tensor.matmul(
                            pu[g * BL:(g + 1) * BL, :], sh,
                            wl_sb[ki][:, g * GW:(g + 1) * GW],
                            start=False, stop=False,
                            tile_position=(0, g * BL))
                for ki in range(8):
                    sl_ = stat(tTl, ki)
                    for g in range(4):
                        nc.tensor.matmul(
                            pu[g * BL:(g + 1) * BL, :], sl_,
                            wh_sb[ki][:, g * GW:(g + 1) * GW],
                            start=False, stop=(ki == 7),
                            tile_position=(0, g * BL))

                y2 = y2s[:, t:t + 1]
                ny2 = ny2s[:, t:t + 1]
                y2p1 = y2p1s[:, t:t + 1]
                By = Bys[:, t:t + 1]
                cyt = cys[:, t:t + 1]
                syb = sybs[:, t:t + 1]

                # --- folded norms: three decoupled partial->reduce paths --
                scr = rb.tile([128, 256], f32, tag="scr")
                prtA = rb.tile([128, 1], f32, tag="prtA")
                nc.scalar.activation(scr[:], pu[:, 0:256], AF.Square,
                                     accum_out=prtA[:])
                prtC = rb.tile([128, 1], f32, tag="prtC")
                nc.vector.tensor_copy(prtC[:], pu[:, 256:257])
                scr2 = rb.tile([128, 256], f32, tag="scr2")
                prtB = rb.tile([128, 1], f32, tag="prtB")
                nc.vector.scalar_tensor_tensor(
                    scr2[:], pu[:, 0:256], 1.0, yt[:],
                    OP.mult, OP.mult, accum_out=prtB[:])
                psmA = rpsS.tile([128, 1], f32, tag="psmA")
                psmX = rpsS.tile([128, 3], f32, tag="psmX")
                nc.tensor.matmul(psmA[:], i4_sb[:], prtA[:],
                                 start=True, stop=True)
                nc.tensor.matmul(psmX[:, 1:2], i4_sb[:], prtC[:],
                                 start=True, stop=True)
                nc.tensor.matmul(psmX[:, 0:1], i4_sb[:], prtB[:],
                                 start=True, stop=True)
                s2u = psmA[:]
                suy = psmX[:, 0:1]
                sub = psmX[:, 1:2]

                # --- X-norm block -----------------------------------------
                lnu = s("lnu")
                nc.scalar.activation(lnu[:], s2u, AF.Ln, bias=1e-12)
                nG = s("nG")
                nc.scalar.activation(nG[:], lnu[:], AF.Exp, scale=0.5)
                ev = s("ev")
                nc.scalar.activation(ev[:], nG[:], AF.Exp, scale=kk2[:])
                rnG = s("rnG")
                nc.scalar.activation(rnG[:], lnu[:], AF.Exp, scale=-0.5)
                q = s("q")
                nc.vector.tensor_scalar(q[:], ev[:], 1.0, None, OP.add)
                rv = s("rv")
                nc.vector.reciprocal(rv[:], q[:])
                th = s("th")
                nc.vector.tensor_scalar(th[:], rv[:], -2.0, 1.0, OP.mult, OP.add)
                nc.vector.tensor_scalar(th[:], th[:], MAXN, None, OP.min)
                dneg = s("dneg")
                nc.vector.scalar_tensor_tensor(
                    dneg[:], rv[:], rv[:], rv[:], OP.mult, OP.subtract)
                B1 = s("B1")
                nc.vector.tensor_scalar(B1[:], dneg[:], -4.0, BMIN,
                                        OP.mult, OP.max)
                cwm = s("cwm")
                nc.vector.tensor_tensor(cwm[:], th[:], rnG[:], OP.mult)
                for db in range(3):
                    nc.tensor.matmul(pT[0:1, 0:256], cwm[0:128, 0:1],
                                     brep_sb[:, 0:256],
                                     start=True, stop=True,
                                     tile_position=(0, 0),
                                     skip_group_check=True)

                # --- madd1 scalars ----------------------------------------
                xy1 = s("xy1")
                nc.vector.tensor_scalar(xy1[:], suy, cyt, cwm[:],
                                        OP.mult, OP.mult)
                r1 = s("r1")
                nc.vector.tensor_scalar(r1[:], suy, cyt, B1[:],
                                        OP.mult, OP.mult)
                A1 = s("A1")
                nc.vector.tensor_scalar(A1[:], xy1[:], 2.0, y2p1,
                                        OP.mult, OP.add)
                den1 = s("den1")
                nc.vector.tensor_scalar(den1[:], B1[:], ny2, A1[:],
                                        OP.mult, OP.add)
                aa = s("aa")
                nc.vector.tensor_tensor(aa[:], A1[:], cwm[:], OP.mult)
                raa = s("raa")
                nc.vector.reciprocal(raa[:], aa[:])
                r1c = s("r1c")
                nc.vector.tensor_scalar(r1c[:], B1[:], cyt, raa[:],
                                        OP.mult, OP.mult)
                r2 = s("r2")
                nc.vector.tensor_scalar(r2[:], aa[:], s2u, None, OP.mult)
                r3 = s("r3")
                nc.vector.scalar_tensor_tensor(
                    r3[:], r1[:], 2.0, r2[:], OP.mult, OP.add)
                r5 = s("r5")
                nc.vector.tensor_scalar(r5[:], B1[:], B1[:], y2,
                                        OP.mult, OP.mult)
                s2n1 = s("s2n1")
                nc.vector.scalar_tensor_tensor(
                    s2n1[:], r3[:], aa[:], r5[:], OP.mult, OP.add)
                rn1 = s("rn1")
                nc.scalar.activation(rn1[:], s2n1[:], AF.Ln, bias=1e-12)
                nc.scalar.activation(rn1[:], rn1[:], AF.Exp, scale=-0.5)
                rd1 = s("rd1")
                nc.vector.reciprocal(rd1[:], den1[:])
                gam = s("gam")
                nc.vector.tensor_scalar(gam[:], rn1[:], MAXN, rd1[:],
                                        OP.mult, OP.min)
                for db in range(3):
                    nc.tensor.matmul(pT[0:1, 0:256], gam[0:128, 0:1],
                                     brep_sb[:, 0:256],
                                     start=True, stop=True,
                                     tile_position=(0, 0),
                                     skip_group_check=True)
                B2 = s("B2")
                nc.vector.tensor_scalar(B2[:], B1[:], By, rd1[:],
                                        OP.mult, OP.mult)
                nc.vector.tensor_scalar(B2[:], B2[:], BMIN, None, OP.max)
                x2b = s("x2b")
                nc.vector.tensor_scalar(x2b[:], B2[:], -1.0, 1.0,
                                        OP.mult, OP.add)

                # --- madd2 scalars (with hyperbolic bias b) ---------------
                t3 = s("t3")
                nc.vector.tensor_scalar(t3[:], aa[:], sub, None, OP.mult)
                t5 = s("t5")
                nc.vector.scalar_tensor_tensor(
                    t5[:], B1[:], syb, t3[:], OP.mult, OP.add)
                xy2d = s("xy2d")
                nc.vector.tensor_scalar(xy2d[:], t5[:], gam[:], 2.0,
                                        OP.mult, OP.mult)
                A2 = s("A2")
                nc.vector.tensor_scalar(A2[:], xy2d[:], opb2c, None, OP.add)
                den2 = s("den2")
                nc.vector.scalar_tensor_tensor(
                    den2[:], B2[:], nb2c, A2[:], OP.mult, OP.add)
                g1 = s("g1")
                nc.vector.tensor_scalar(g1[:], A2[:], gam[:], None, OP.mult)
                rg1 = s("rg1")
                nc.vector.reciprocal(rg1[:], g1[:])
                r2c = s("r2c")
                nc.vector.tensor_scalar(r2c[:], B2[:], raa[:], rg1[:],
                                        OP.mult, OP.mult)
                W1 = rb.tile([128, 256], f32, tag="W1")
                nc.vector.scalar_tensor_tensor(
                    W1[:], yt[:], r1c[:], pu[:, 0:256], OP.mult, OP.add)
                W2 = rb.tile([128, 256], f32, tag="W2")
                nc.vector.scalar_tensor_tensor(
                    W2[:], brep_sb[:], r2c[:], W1[:], OP.mult, OP.add)
                u1 = s("u1")
                nc.vector.tensor_tensor(u1[:], A2[:], x2b[:], OP.mult)
                u3 = s("u3")
                nc.vector.scalar_tensor_tensor(
                    u3[:], xy2d[:], B2[:], u1[:], OP.mult, OP.add)
                u5 = s("u5")
                nc.vector.tensor_scalar(u5[:], B2[:], B2[:], b2c,
                                        OP.mult, OP.mult)
                s2n2 = s("s2n2")
                nc.vector.scalar_tensor_tensor(
                    s2n2[:], u3[:], A2[:], u5[:], OP.mult, OP.add)
                ln2 = s("ln2")
                nc.scalar.activation(ln2[:], s2n2[:], AF.Ln, bias=1e-12)
                n2r = s("n2r")
                nc.scalar.activation(n2r[:], ln2[:], AF.Exp, scale=0.5)
                rd2 = s("rd2")
                nc.vector.reciprocal(rd2[:], den2[:])
                Bz2 = s("Bz2")
                nc.vector.tensor_scalar(Bz2[:], B2[:], Bbc, rd2[:],
                                        OP.mult, OP.mult)
                nc.vector.tensor_scalar(Bz2[:], Bz2[:], BMIN, None, OP.max)
                lnB = s("lnB")
                nc.scalar.activation(lnB[:], Bz2[:], AF.Ln)
                n2 = s("n2")
                nc.vector.tensor_scalar(n2[:], n2r[:], rd2[:], MAXN,
                                        OP.mult, OP.min)
                for db in range(3):
                    nc.tensor.matmul(pT[0:1, 0:256], n2[0:128, 0:1],
                                     brep_sb[:, 0:256],
                                     start=True, stop=True,
                                     tile_position=(0, 0),
                                     skip_group_check=True)
                lnA = s("lnA")
                nc.scalar.activation(lnA[:], n2[:], AF.Ln, bias=1.0)
                rn2r = s("rn2r")
                nc.scalar.activation(rn2r[:], ln2[:], AF.Exp, scale=-0.5)
                a2 = s("a2")
                nc.vector.scalar_tensor_tensor(
                    a2[:], lnA[:], 2.0, lnB[:], OP.mult, OP.subtract)
                L = s("L")
                nc.vector.tensor_scalar(L[:], a2[:], rn2r[:], 0.5,
                                        OP.mult, OP.mult)
                cut = s("cut")
                nc.vector.tensor_scalar(cut[:], L[:], g1[:], aa[:],
                                        OP.mult, OP.mult)

                # --- wide state update: W2 = uG + (cycp/cu)y + (cb/cu)b
                # was built mid-chain; apply cu via the tanh scale slot ----
                tv = rb.tile([128, 256], f32, tag="tv")
                nc.scalar.activation(tv[:], W2[:], AF.Tanh, scale=cut[:])
                s2tp = rb.tile([128, 1], f32, tag="s2tp")
                nc.scalar.activation(scr[:], tv[:], AF.Square,
                                     accum_out=s2tp[:])
                nc.tensor.matmul(psmX[:, 2:3], i4_sb[:], s2tp[:],
                                 start=True, stop=True)
                rnt = s("rnt")
                nc.scalar.activation(rnt[:], psmX[:, 2:3], AF.Ln,
                                     bias=1e-12)
                nc.scalar.activation(rnt[:], rnt[:], AF.Exp, scale=-0.5)
                nc.vector.tensor_scalar(kk2[:], rnt[:], 2.0 * _A999, 2.0,
                                        OP.mult, OP.min)

                # masked last-state accumulate (folded)
                nc.vector.scalar_tensor_tensor(
                    tsel[:], tv[:], eqm_sb[:, t:t + 1], tsel[:],
                    OP.mult, OP.add)
                nc.vector.scalar_tensor_tensor(
                    s2tsel[:], psmX[0:BL, 2:3], eqm_sb[0:BL, t:t + 1],
                    s2tsel[:], OP.mult, OP.add)

                # --- transpose new state (2 full 128x128 blocks) ----------
                nc.tensor.transpose(pT[:, 0:128], tv[:, 0:128], ident_sb[:])
                nc.tensor.transpose(pT[:, 128:256], tv[:, 128:256],
                                    ident_sb[:])
                nc.vector.tensor_copy(tTh[:], pT[:])
                nc.vector.tensor_tensor(tTl[:], pT[:], tTh[:], OP.subtract)

                # inject one deferred P1 tile every 4th step, at the END
                # of the body so its DVE/PE work queues into the next
                # chain window instead of ahead of this step's reduce ops
                # (tile j lands at step 4(j-8)+1 = 4j-31, consumed at 4j)
                if next_tile < NTILES and t % 4 == 1:
                    p1_tile(next_tile)
                    next_tile += 1
                    if next_tile % 8 == 0 or next_tile == NTILES:
                        p1_chain(next_chain, next_tile)
                        next_chain = next_tile

        # =============== P3: epilogue =======================================
        with (
            tc.tile_pool(name="ebig", bufs=1) as eb,
            tc.tile_pool(name="eps", bufs=1, space="PSUM") as eps,
            tc.tile_pool(name="esc", bufs=2) as es,
        ):
            def e(tag):
                return es.tile([BL, 1], f32, tag=tag, name=tag)

            # last state scalars: ntl = |t_last|, tanh, B_h
            ntl = e("ntl")
            nc.scalar.activation(ntl[:], s2tsel[:], AF.Ln, bias=1e-12)
            nc.scalar.activation(ntl[:], ntl[:], AF.Exp, scale=0.5)
            rntl = e("rntl")
            nc.vector.reciprocal(rntl[:], ntl[:])
            evl = e("evl")
            nc.scalar.activation(evl[:], ntl[:], AF.Exp, scale=2.0)
            nc.vector.tensor_scalar(evl[:], evl[:], 1.0, None, OP.add)
            rq = e("rq")
            nc.vector.reciprocal(rq[:], evl[:])
            tnl = e("tnl")
            nc.vector.tensor_scalar(tnl[:], rq[:], -2.0, 1.0, OP.mult, OP.add)
            nc.vector.tensor_scalar(tnl[:], tnl[:], MAXN, None, OP.min)
            dnl = e("dnl")
            nc.vector.scalar_tensor_tensor(
                dnl[:], rq[:], rq[:], rq[:], OP.mult, OP.subtract)
            Bh = e("Bh")
            nc.vector.tensor_scalar(Bh[:], dnl[:], -4.0, BMIN, OP.mult, OP.max)
            mul_ = e("mul_")
            nc.vector.tensor_tensor(mul_[:], tnl[:], rntl[:], OP.mult)
            kk2l = e("kk2l")
            nc.vector.tensor_scalar(kk2l[:], rntl[:], 2.0 * _A999, 2.0,
                                    OP.mult, OP.min)

            # transpose folded t_last, hyperbolic linear layer
            pT = eps.tile([128, 256], f32, tag="epT")
            nc.tensor.transpose(pT[:, 0:128], tsel[:, 0:128], ident_sb[:])
            nc.tensor.transpose(pT[:, 128:256], tsel[:, 128:256], ident_sb[:])
            tselT = eb.tile([128, 256], f32r, tag="tselT")
            nc.vector.tensor_copy(tselT[:], pT[:])
            wl_sb = []
            for ki in range(8):
                t_ = eb.tile([128, HP1], f32r, tag=f"wl{ki}", name=f"wl{ki}")
                nc.sync.dma_start(t_[:], Wl[ki * 128:(ki + 1) * 128, :])
                wl_sb.append(t_)
            pl = eps.tile([BL, HP1], f32, tag="pl")
            for ki in range(8):
                c0 = (ki % 2) * 128 + (ki // 2) * BL
                for (n0, n1) in ((0, 512), (512, 1024), (1024, 1026)):
                    nc.tensor.matmul(
                        pl[:, n0:n1],
                        tselT[:, c0:c0 + BL],
                        wl_sb[ki][:, n0:n1],
                        start=(ki == 0), stop=(ki == 7))
            scrL = eb.tile([BL, H], f32, tag="scrL")
            s2u = e("es2u")
            nc.scalar.activation(scrL[:], pl[:, 0:H], AF.Square,
                                 accum_out=s2u[:])
            sub = e("esub")
            nc.vector.tensor_copy(sub[:], pl[:, H:H + 1])
            # X = mmatvec(h_last, Wl) folded scalars
            nG = e("enG")
            nc.scalar.activation(nG[:], s2u[:], AF.Ln, bias=1e-12)
            nc.scalar.activation(nG[:], nG[:], AF.Exp, scale=0.5)
            evx = e("evx")
            nc.scalar.activation(evx[:], nG[:], AF.Exp, scale=kk2l[:])
            nc.vector.tensor_scalar(evx[:], evx[:], 1.0, None, OP.add)
            rvx = e("rvx")
            nc.vector.reciprocal(rvx[:], evx[:])
            th = e("eth")
            nc.vector.tensor_scalar(th[:], rvx[:], -2.0, 1.0, OP.mult, OP.add)
            nc.vector.tensor_scalar(th[:], th[:], MAXN, None, OP.min)
            dnx = e("ednx")
            nc.vector.scalar_tensor_tensor(
                dnx[:], rvx[:], rvx[:], rvx[:], OP.mult, OP.subtract)
            B1 = e("eB1")
            nc.vector.tensor_scalar(B1[:], dnx[:], -4.0, BMIN, OP.mult, OP.max)
            x2 = e("ex2")
            nc.vector.tensor_tensor(x2[:], th[:], th[:], OP.mult)
            cwm = e("ecwm")
            nc.vector.reciprocal(cwm[:], nG[:])
            nc.vector.tensor_tensor(cwm[:], cwm[:], th[:], OP.mult)
            # madd(X, bc): y-side is the constant bias point bc
            bc2 = cst_sb[0:BL, C_BC2:C_BC2 + 1]
            Bbc2 = cst_sb[0:BL, C_BBC:C_BBC + 1]
            subt = e("esubt")
            nc.vector.tensor_tensor(subt[:], sub[:], cwm[:], OP.mult)
            A1 = e("eA1")
            nc.vector.tensor_scalar(A1[:], subt[:], 2.0, 1.0, OP.mult, OP.add)
            nc.vector.tensor_scalar(A1[:], A1[:], bc2, None, OP.add)
            ones32e = e("ones32e")
            nc.vector.memset(ones32e[:], 1.0)
            den = e("eden")
            nc.vector.scalar_tensor_tensor(
                den[:], x2[:], bc2, ones32e[:], OP.mult, OP.add)
            nc.vector.scalar_tensor_tensor(
                den[:], subt[:], 2.0, den[:], OP.mult, OP.add)
            aa = e("eaa")
            nc.vector.tensor_tensor(aa[:], A1[:], cwm[:], OP.mult)
            p1 = e("ep1")
            nc.vector.tensor_tensor(p1[:], aa[:], aa[:], OP.mult)
            nc.vector.tensor_tensor(p1[:], p1[:], s2u[:], OP.mult)
            p2 = e("ep2")
            nc.vector.tensor_tensor(p2[:], aa[:], B1[:], OP.mult)
            nc.vector.tensor_tensor(p2[:], p2[:], sub[:], OP.mult)
            nc.vector.tensor_scalar(p2[:], p2[:], 2.0, None, OP.mult)
            p3 = e("ep3")
            nc.vector.tensor_scalar(p3[:], B1[:], B1[:], bc2, OP.mult, OP.mult)
            s2n = e("es2n")
            nc.vector.tensor_tensor(s2n[:], p1[:], p2[:], OP.add)
            nc.vector.tensor_tensor(s2n[:], s2n[:], p3[:], OP.add)
            n1 = e("en1")
            nc.scalar.activation(n1[:], s2n[:], AF.Ln, bias=1e-12)
            nc.scalar.activation(n1[:], n1[:], AF.Exp, scale=0.5)
            rd = e("erd")
            nc.vector.reciprocal(rd[:], den[:])
            gam = e("egam")
            nc.vector.reciprocal(gam[:], n1[:])
            nc.vector.tensor_scalar(gam[:], gam[:], MAXN, None, OP.mult)
            nc.vector.tensor_tensor(gam[:], gam[:], rd[:], OP.min)
            Brepr = e("eBrepr")
            nc.vector.tensor_scalar(Brepr[:], B1[:], Bbc2, rd[:],
                                    OP.mult, OP.mult)
            nc.vector.tensor_scalar(Brepr[:], Brepr[:], BMIN, None, OP.max)
            x2r = e("ex2r")
            nc.vector.tensor_scalar(x2r[:], Brepr[:], -1.0, 1.0,
                                    OP.mult, OP.add)
            c1 = e("ec1")
            nc.vector.tensor_tensor(c1[:], gam[:], A1[:], OP.mult)
            nc.vector.tensor_tensor(c1[:], c1[:], cwm[:], OP.mult)
            c2 = e("ec2")
            nc.vector.tensor_tensor(c2[:], gam[:], B1[:], OP.mult)
            rep = eb.tile([BL, H], f32, tag="rep")
            nc.scalar.activation(rep[:], pl[:, 0:H], AF.Copy, scale=c1[:])
            rep2 = eb.tile([BL, H], f32, tag="rep2")
            nc.vector.scalar_tensor_tensor(
                rep2[:], bcrep_sb[0:BL, :], c2[:], rep[:], OP.mult, OP.add)

            pk = eb.tile([BL, HP2], f32, tag="pk")
            nc.vector.tensor_copy(pk[:, 0:H], rep2[:])
            nc.vector.tensor_copy(pk[:, H:H + 1], x2r[:])
            nc.vector.tensor_copy(pk[:, H + 1:H + 2], Brepr[:])
            nc.sync.dma_start(ccin[:, :], pk[:])
            nc.gpsimd.collective_compute(
                "AllGather", mybir.AluOpType.bypass,
                replica_groups=[list(range(8))],
                ins=[ccin[:, :]], outs=[ccout[:, :]])
            # static full-batch slices: premise rows 0:128, hypothesis 128:256
            ccp = eb.tile([128, HP2], f32, tag="ccp")
            nc.sync.dma_start(ccp[:], ccout[0:128, :])
            cch = eb.tile([128, HP2], f32, tag="cch")
            nc.sync.dma_start(cch[:], ccout[128:256, :])
            prep = ccp[0:128, 0:H]
            hrep = cch[0:128, 0:H]
            px2 = ccp[0:128, H:H + 1]
            hy2c = cch[0:128, H:H + 1]
            pB = ccp[0:128, H + 1:H + 2]
            hB = cch[0:128, H + 1:H + 2]
            ones128 = eb.tile([128, 1], f32, tag="ones128")
            nc.vector.memset(ones128[:], 1.0)

            def f(tag):
                return es.tile([128, 1], f32, tag=tag, name=tag)

            # combine: rep = madd(p_rep, h_rep)
            xyf = f("xyf")
            scrF = eb.tile([128, H], f32, tag="scrF")
            nc.vector.scalar_tensor_tensor(
                scrF[:], prep, 1.0, hrep, OP.mult, OP.mult, accum_out=xyf[:])
            Af = f("Af")
            nc.vector.tensor_scalar(Af[:], xyf[:], 2.0, 1.0, OP.mult, OP.add)
            nc.vector.tensor_scalar(Af[:], Af[:], hy2c, None, OP.add)
            Bf = f("Bf")
            nc.vector.tensor_scalar(Bf[:], px2, -1.0, 1.0, OP.mult, OP.add)
            denf = f("denf")
            nc.vector.scalar_tensor_tensor(
                denf[:], px2, hy2c, ones128[:], OP.mult, OP.add)
            nc.vector.scalar_tensor_tensor(
                denf[:], xyf[:], 2.0, denf[:], OP.mult, OP.add)
            numf = eb.tile([128, H], f32, tag="numf")
            nc.scalar.activation(numf[:], prep, AF.Copy, scale=Af[:])
            numf2 = eb.tile([128, H], f32, tag="numf2")
            nc.vector.scalar_tensor_tensor(
                numf2[:], hrep, Bf[:], numf[:], OP.mult, OP.add)
            s2f = f("s2f")
            nc.scalar.activation(scrF[:], numf2[:], AF.Square, accum_out=s2f[:])
            nf = f("nf")
            nc.scalar.activation(nf[:], s2f[:], AF.Ln, bias=1e-12)
            nc.scalar.activation(nf[:], nf[:], AF.Exp, scale=0.5)
            rdf = f("rdf")
            nc.vector.reciprocal(rdf[:], denf[:])
            gf = f("gf")
            nc.vector.reciprocal(gf[:], nf[:])
            nc.vector.tensor_scalar(gf[:], gf[:], MAXN, None, OP.mult)
            nc.vector.tensor_tensor(gf[:], gf[:], rdf[:], OP.min)
            Brf = f("Brf")
            nc.vector.tensor_scalar(Brf[:], pB, hB, rdf[:], OP.mult, OP.mult)
            nc.vector.tensor_scalar(Brf[:], Brf[:], BMIN, None, OP.max)
            y2f = f("y2f")
            nc.vector.tensor_scalar(y2f[:], Brf[:], -1.0, 1.0, OP.mult, OP.add)
            repf = eb.tile([128, H], f32, tag="repf")
            nc.scalar.activation(repf[:], numf2[:], AF.Copy, scale=gf[:])

            # MLR
            pT2 = eps.tile([128, H], f32, tag="epT2")
            for c in range(8):
                nc.tensor.transpose(
                    pT2[:, c * 128:(c + 1) * 128],
                    repf[:, c * 128:(c + 1) * 128],
                    ident_sb[:])
            repT = eb.tile([128, H], f32r, tag="repT")
            nc.vector.tensor_copy(repT[:], pT2[:])
            mlr_sb = []
            for ki in range(8):
                t_ = eb.tile([128, 2 * NCLS], f32r, tag=f"mlr{ki}",
                             name=f"mlr{ki}")
                nc.sync.dma_start(t_[:], mlrT[ki * 128:(ki + 1) * 128, :])
                mlr_sb.append(t_)
            pm = eps.tile([128, 2 * NCLS], f32, tag="pm")
            for ki in range(8):
                nc.tensor.matmul(
                    pm[:, :],
                    repT[:, ki * 128:(ki + 1) * 128],
                    mlr_sb[ki][:, :],
                    start=(ki == 0), stop=(ki == 7))

            def e3(tag):
                return es.tile([128, NCLS], f32, tag=tag, name=tag)

            xp = e3("xp")
            nc.vector.tensor_copy(xp[:], pm[:, 0:NCLS])
            xa = e3("xa")
            nc.vector.tensor_copy(xa[:], pm[:, NCLS:2 * NCLS])
            pk2 = cst_sb[:, C_PK2:C_PK2 + NCLS]
            pkak = cst_sb[:, C_PKAK:C_PKAK + NCLS]
            anc = cst_sb[:, C_AN:C_AN + NCLS]
            lamc = cst_sb[:, C_LAM:C_LAM + NCLS]
            Bpk = cst_sb[:, C_BPK:C_BPK + NCLS]
            xyk = e3("xyk")
            nc.vector.tensor_scalar(xyk[:], xp[:], -1.0, None, OP.mult)
            Ak = e3("Ak")
            nc.vector.tensor_scalar(Ak[:], xyk[:], 2.0, 1.0, OP.mult, OP.add)
            nc.vector.tensor_scalar(Ak[:], Ak[:], y2f[:], None, OP.add)
            Bk = e3("Bk")
            nc.vector.tensor_scalar(Bk[:], pk2, -1.0, 1.0, OP.mult, OP.add)
            denk = e3("denk")
            nc.vector.tensor_scalar(denk[:], pk2, y2f[:], None, OP.mult)
            nc.vector.tensor_tensor(denk[:], denk[:], xyk[:], OP.add)
            nc.vector.tensor_tensor(denk[:], denk[:], xyk[:], OP.add)
            nc.vector.tensor_scalar(denk[:], denk[:], 1.0, None, OP.add)
            q1k = e3("q1k")
            nc.vector.tensor_tensor(q1k[:], Ak[:], Ak[:], OP.mult)
            nc.vector.tensor_tensor(q1k[:], q1k[:], pk2, OP.mult)
            q2k = e3("q2k")
            nc.vector.tensor_tensor(q2k[:], Ak[:], Bk[:], OP.mult)
            nc.vector.tensor_tensor(q2k[:], q2k[:], xyk[:], OP.mult)
            nc.vector.tensor_scalar(q2k[:], q2k[:], 2.0, None, OP.mult)
            q3k = e3("q3k")
            nc.vector.tensor_tensor(q3k[:], Bk[:], Bk[:], OP.mult)
            nc.vector.tensor_scalar(q3k[:], q3k[:], y2f[:], None, OP.mult)
            s2k = e3("s2k")
            nc.vector.tensor_tensor(s2k[:], q1k[:], q2k[:], OP.add)
            nc.vector.tensor_tensor(s2k[:], s2k[:], q3k[:], OP.add)
            n1k = e3("n1k")
            nc.scalar.activation(n1k[:], s2k[:], AF.Ln, bias=1e-12)
            nc.scalar.activation(n1k[:], n1k[:], AF.Exp, scale=0.5)
            rdk = e3("rdk")
            nc.vector.reciprocal(rdk[:], denk[:])
            gk = e3("gk")
            nc.vector.reciprocal(gk[:], n1k[:])
            nc.vector.tensor_scalar(gk[:], gk[:], MAXN, None, OP.mult)
            nc.vector.tensor_tensor(gk[:], gk[:], rdk[:], OP.min)
            # 1 - |z|^2 via identity: Bz = max(Bpk * Brf * rdk, BMIN)
            Bzk = e3("Bzk")
            nc.vector.tensor_scalar(Bzk[:], Bpk, Brf[:], None, OP.mult)
            nc.vector.tensor_tensor(Bzk[:], Bzk[:], rdk[:], OP.mult)
            nc.vector.tensor_scalar(Bzk[:], Bzk[:], BMIN, None, OP.max)
            zak = e3("zak")
            nc.vector.tensor_tensor(zak[:], Ak[:], pkak, OP.mult)
            nc.vector.tensor_scalar(zak[:], zak[:], -1.0, None, OP.mult)
            q4k = e3("q4k")
            nc.vector.tensor_tensor(q4k[:], Bk[:], xa[:], OP.mult)
            nc.vector.tensor_tensor(zak[:], zak[:], q4k[:], OP.add)
            nc.vector.tensor_tensor(zak[:], zak[:], gk[:], OP.mult)
            vk = e3("vk")
            nc.vector.tensor_tensor(vk[:], Bzk[:], anc, OP.mult)
            nc.vector.reciprocal(vk[:], vk[:])
            nc.vector.tensor_tensor(vk[:], vk[:], zak[:], OP.mult)
            nc.vector.tensor_scalar(vk[:], vk[:], 2.0, None, OP.mult)
            # asinh(v) = sign(v) * ln(|v| + sqrt(v^2+1))  (cancellation-free)
            av = e3("av")
            nc.scalar.activation(av[:], vk[:], AF.Abs)
            sg = e3("sg")
            nc.scalar.activation(sg[:], vk[:], AF.Sign)
            sq = e3("sq")
            nc.vector.tensor_tensor(sq[:], vk[:], vk[:], OP.mult)
            nc.vector.tensor_scalar(sq[:], sq[:], 1.0, None, OP.add)
            nc.scalar.activation(sq[:], sq[:], AF.Ln)
            nc.scalar.activation(sq[:], sq[:], AF.Exp, scale=0.5)
            nc.vector.tensor_tensor(sq[:], sq[:], av[:], OP.add)
            nc.scalar.activation(sq[:], sq[:], AF.Ln)
            nc.vector.tensor_tensor(sq[:], sq[:], sg[:], OP.mult)
            logit = e3("logit")
            nc.vector.tensor_tensor(logit[:], sq[:], anc, OP.mult)
            nc.vector.tensor_tensor(logit[:], logit[:], lamc, OP.mult)
            nc.sync.dma_start(out[:, :], logit[:])

    nc.finalize()
    return nc


def _bf16_pair(x):
    """Split fp32 array into (hi, lo) bf16 pieces: hi = bf16(x),
    lo = bf16(x - hi)."""
    from concourse import mybir
    bd = mybir.dt.np(mybir.dt.bfloat16)
    hi = x.astype(np.float32).astype(bd)
    lo = (x.astype(np.float32) - hi.astype(np.float32)).astype(bd)
    return hi, lo


def _host_prep(inputs, s_steps=S):
    f = np.float32
    emb = inputs["emb_table"].astype(f)
    in_maps = []
    for c in range(8):
        q, r = c % 4, c // 4
        ids = (inputs["premise"] if r == 0 else inputs["hypothesis"])[
            q * BL:(q + 1) * BL, :s_steps].astype(np.int64)
        lens = (inputs["p_len"] if r == 0 else inputs["h_len"])[
            q * BL:(q + 1) * BL].astype(np.int64)
        lens = np.minimum(lens, s_steps)
        U = (inputs["Up"] if r == 0 else inputs["Uh"]).astype(f)
        W = (inputs["Wp"] if r == 0 else inputs["Wh"]).astype(f)
        bv = (inputs["bp"] if r == 0 else inputs["bh"]).astype(f)
        Wlin = (inputs["Wcp"] if r == 0 else inputs["Wch"]).astype(f)
        bc = (inputs["bcp"] if r == 0 else inputs["bch"]).astype(f)
        a_mlr = inputs["a_mlr"].astype(f)
        p_mlr = inputs["p_mlr"].astype(f)

        x = emb[ids]
        xTv = np.ascontiguousarray(x.transpose(2, 1, 0).reshape(E, s_steps * BL))
        xTh_v, xTl_v = _bf16_pair(xTv)
        xn2 = np.maximum((x.astype(np.float64) ** 2).sum(-1), 1e-12)
        xn2v = np.ascontiguousarray(
            xn2.transpose(1, 0).reshape(-1, 128).T).astype(f)
        zc = np.zeros((H, 1), f)
        zc2 = np.zeros((H, 2), f)
        Ucv = np.concatenate([U, (U @ bv)[:, None], np.zeros((E, 1), f)], 1)
        # W packed per col-group g: [W[:,256g:256g+256] | bcol | pad] where
        # bcol = W@b only in group 3, zeros elsewhere.
        Wb = (W @ bv)[:, None]
        Wrv = np.concatenate(
            [W[:, 0:256], zc2, W[:, 256:512], zc2, W[:, 512:768], zc2,
             W[:, 768:1024], Wb, zc], 1)
        Wlv = np.concatenate([Wlin, (Wlin @ bc)[:, None], zc], 1)
        Uch_v, Ucl_v = _bf16_pair(Ucv)
        Wrh_v, Wrl_v = _bf16_pair(Wrv)
        b2 = float(bv.astype(np.float64) @ bv.astype(np.float64))
        bc2 = float(bc.astype(np.float64) @ bc.astype(np.float64))
        p2 = np.sum(p_mlr.astype(np.float64) ** 2, -1)
        cstv = np.zeros((128, CSTW), f)
        cstv[:, C_OPB2] = 1.0 + b2
        cstv[:, C_NB2] = -b2
        cstv[:, C_B2] = b2
        cstv[:, C_BB] = 1.0 - b2
        cstv[:, C_BC2] = bc2
        cstv[:, C_BBC] = 1.0 - bc2
        cstv[:, C_PK2:C_PK2 + NCLS] = p2
        cstv[:, C_PKAK:C_PKAK + NCLS] = np.sum(
            p_mlr.astype(np.float64) * a_mlr.astype(np.float64), -1)
        cstv[:, C_AN:C_AN + NCLS] = np.sqrt(
            np.maximum(np.sum(a_mlr.astype(np.float64) ** 2, -1), 1e-12))
        cstv[:, C_LAM:C_LAM + NCLS] = 2.0 / np.maximum(1.0 - p2, 1e-5)
        cstv[:, C_BPK:C_BPK + NCLS] = 1.0 - p2
        eqv = (np.arange(s_steps)[None, :] == (lens - 1)[:, None]).astype(f)
        brep_f = np.tile(bv.reshape(4, 256)[:, None, :],
                         (1, BL, 1)).reshape(128, 256).astype(f)
        in_maps.append({
            "xTh": xTh_v, "xTl": xTl_v, "xn2d": xn2v,
            "Uch": Uch_v, "Ucl": Ucl_v, "Wrh": Wrh_v, "Wrl": Wrl_v,
            "Wl": Wlv,
            "brep": brep_f,
            "bcrep": np.broadcast_to(bc, (128, H)).copy(),
            "mlrT": np.concatenate([p_mlr.T, a_mlr.T], 1).astype(f),
            "cst": cstv, "eqm": np.tile(eqv, (4, 1)).astype(f),
            "ident": np.eye(128, dtype=f),
            "i4m": np.tile(np.eye(BL, dtype=f), (4, 4)),
        })
    return in_maps


def kernel(premise, p_len, hypothesis, h_len, emb_table, Wp, Up, bp,
           Wh, Uh, bh, Wcp, bcp, Wch, bch, a_mlr, p_mlr,
           s_steps=S, trace=False):
    from concourse.bass_utils import run_bass_kernel_spmd
    inputs = dict(premise=premise, p_len=p_len, hypothesis=hypothesis,
                  h_len=h_len, emb_table=emb_table, Wp=Wp, Up=Up, bp=bp,
                  Wh=Wh, Uh=Uh, bh=bh, Wcp=Wcp, bcp=bcp, Wch=Wch, bch=bch,
                  a_mlr=a_mlr, p_mlr=p_mlr)
    inputs = {k: np.asarray(v) for k, v in inputs.items()}
    if s_steps not in _CACHE:
        _CACHE[s_steps] = _build(s_steps)
    nc = _CACHE[s_steps]
    in_maps = _host_prep(inputs, s_steps)
    res = run_bass_kernel_spmd(nc, in_maps, core_ids=list(range(8)),
                               trace=trace)
    kernel.last_results = res
    return res.results[0]["out"].astype(np.float32)


kernel.last_results = None

